# revision 12
# baseline (speedup 1.0000x reference)
"""Trainium2 Bass kernel for nn_DecoderStack — v4.

Sharding over 8 NeuronCores: core c -> batch b=c//2, half h=c%2.
MHA1/MHA2: head-split (8 heads/core), pair ReduceScatter after Wo (chunked
4x[256,D] bf16, pipelined with tails).  FFN: t-split (full FF=4096 over the
core's own 512 rows), entirely bf16 — no FFN collective.

fp8e4 DoubleRow matmuls: QKV projections, attnV (V pre-scaled by 1/denom),
Wo.  bf16: scores (K=64) and the whole FFN (fp8 there costs too much
accuracy).  Dequant scales are exact powers of two folded into the Exp
activation scale and PSUM-eviction multiplies.

Engine split: PE matmuls; scalar exp (the head-loop pacer) + sqrt; vector
qk/v/pt/wvp evictions + tails; pool (gpsimd) wo/h1/ffp evictions.

Schedule (PE kept dense to hold the 2.4GHz p-state):
  P0: mha1 Q/K/V projection units
  P1: mha1 head loop; filler: 3 mha2-projection units + 1 FFN-in chunk/head
  P2: Wo1 tiles -> chunked RS0 ; FFN-in chunks fc=8..31
  P3: mha2 head loop; filler: FFN-out quarters (wot groups 0-1) + tail1
  P4: Wo2 tiles (k,k+4) -> RS1 chunk k ; FFN-out groups 2-3 between chunks;
      fused tail2+tail3 per chunk
"""

import sys

for _p in ("/opt/trn_rl_repo", "/root/.axon_site"):
    if _p not in sys.path:
        sys.path.insert(0, _p)

import contextlib
import math

import numpy as np

import concourse.bass as bass
import concourse.bacc as bacc
import concourse.tile as tile
from concourse import mybir
from concourse.bass_utils import run_bass_kernel_spmd

B, T, D, H, DK, DV, FF = 4, 1024, 1024, 16, 64, 64, 4096
P = 128
FP32 = mybir.dt.float32
BF16 = mybir.dt.bfloat16
FP8 = mybir.dt.float8e4
NPBF16 = mybir.dt.np(BF16)
NPF8 = mybir.dt.np(FP8)
DR = mybir.MatmulPerfMode.DoubleRow

# quantization scales (exact powers of two)
S_Y = 32.0          # y, x inputs (sigma ~1)
S_ATT = 4096.0      # attn-partial psum scale (baked into wvp)
S_PT = 32.0         # pt fp8 scale


class Cfg:
    def __init__(self, T_=T, D_=D, FF_=FF):
        self.T = T_
        self.D = D_
        self.FF = FF_
        self.NT = T_ // P          # 8 t/s tiles
        self.ND = D_ // P          # 8 d chunks
        self.HT = T_ // 2 // P     # 4 own-row tiles
        self.NF = FF_ // P         # 32 f chunks (full FF)
        self.NG = 4                # wot fc-groups
        self.GF = self.NF // self.NG   # 8 fc per group
        self.HK = 8 * DK           # 512
        self.HV = 8 * DV           # 512
        self.TH = T_ // 2          # 512 own rows


def build_program(cfg: Cfg, scales: dict, n_cores: int = 8, compile: bool = True):
    nc = bacc.Bacc("TRN2", target_bir_lowering=False, debug=False,
                   num_devices=n_cores)
    NT, ND, NF, HT, TH = cfg.NT, cfg.ND, cfg.NF, cfg.HT, cfg.TH
    NG, GF = cfg.NG, cfg.GF
    Tq, DN = cfg.T, cfg.D
    NTH = 2
    NDH = DN // TH

    EXP_SCALE = (1.0 / (S_Y * S_Y * scales["wq1"] * scales["wk1"]),
                 1.0 / (S_Y * S_Y * scales["wq2"] * scales["wk2"]))
    ATT_M = (S_ATT / (S_Y * scales["wv1"]), S_ATT / (S_Y * scales["wv2"]))
    PT_EVICT = S_PT / S_ATT
    WO_EVICT = (1.0 / (S_PT * scales["wo1"]), 1.0 / (S_PT * scales["wo2"]))

    def dram_in(name, shape, dt=FP8):
        return nc.dram_tensor(name, shape, dt, kind="ExternalInput")

    yT = dram_in("yT", [P, ND, Tq])
    yTo = dram_in("yTo", [P, ND, TH], BF16)    # own t-half, true scale
    xT = dram_in("xT", [P, ND, Tq])
    ynat = dram_in("ynat", [TH, DN], FP32)
    wq1 = dram_in("wq1", [P, ND, cfg.HK])
    wk1 = dram_in("wk1", [P, ND, cfg.HK])
    wv1 = dram_in("wv1", [P, ND, cfg.HV])
    wo1 = dram_in("wo1", [P, 4, DN])
    wq2 = dram_in("wq2", [P, ND, cfg.HK])
    wk2 = dram_in("wk2", [P, ND, cfg.HK])
    wv2 = dram_in("wv2", [P, ND, cfg.HV])
    wo2 = dram_in("wo2", [P, 4, DN])
    wi = dram_in("wi", [NF, P, ND, P], BF16)   # W_in chunks, lhsT[d, f]
    wot = dram_in("wot", [P, NF, DN], BF16)
    bi = dram_in("bi", [P, NF], FP32)
    bo = dram_in("bo", [1, DN], FP32)
    out = nc.dram_tensor("out", [TH, DN], FP32, kind="ExternalOutput")

    with tile.TileContext(nc) as tc:
        with contextlib.ExitStack() as ctx:
            p1 = ctx.enter_context(tc.tile_pool(name="p1", bufs=1))
            xw = ctx.enter_context(tc.tile_pool(name="xw", bufs=2))
            qkt = ctx.enter_context(tc.tile_pool(name="qkt", bufs=2))
            ptp = ctx.enter_context(tc.tile_pool(name="ptp", bufs=1))
            expp = ctx.enter_context(tc.tile_pool(name="expp", bufs=4))
            wvpp = ctx.enter_context(tc.tile_pool(name="wvpp", bufs=2))
            wic = ctx.enter_context(tc.tile_pool(name="wic", bufs=2))
            rows = ctx.enter_context(tc.tile_pool(name="rows", bufs=2))
            small = ctx.enter_context(tc.tile_pool(name="small", bufs=2))
            psum = ctx.enter_context(tc.tile_pool(name="psum", bufs=2, space="PSUM"))
            psc = ctx.enter_context(tc.tile_pool(name="psc", bufs=2, space="PSUM"))
            ppp = ctx.enter_context(tc.tile_pool(name="ppp", bufs=1, space="PSUM"))
            dram = ctx.enter_context(tc.tile_pool(name="dram", bufs=1, space="DRAM"))

            # chunked RS staging: RS0: 2x in [512, DN]; RS1: 4x in [256, DN]
            bnc_in = [[dram.tile([4 * P, DN], BF16, tag=f"bi0{k}",
                                 name=f"bi0{k}") for k in range(2)],
                      [dram.tile([2 * P, DN], BF16, tag=f"bi1{k}",
                                 name=f"bi1{k}") for k in range(HT)]]
            bnc_out = [[dram.tile([2 * P, DN], BF16, tag=f"bo0{k}",
                                  name=f"bo0{k}") for k in range(2)],
                       [dram.tile([P, DN], BF16, tag=f"bo1{k}",
                                  name=f"bo1{k}") for k in range(HT)]]

            # ---------------- persistent loads (order matters for startup)
            yT_sb = p1.tile([P, ND, Tq], FP8, tag="yT")
            nc.sync.dma_start(yT_sb[:], yT[:])
            wq1_sb = p1.tile([P, ND, cfg.HK], FP8, tag="wq1")
            wk1_sb = p1.tile([P, ND, cfg.HK], FP8, tag="wk1")
            for pr in range(4):
                nc.sync.dma_start(wq1_sb[:, :, pr * P:(pr + 1) * P],
                                  wq1[:, :, pr * P:(pr + 1) * P])
                nc.sync.dma_start(wk1_sb[:, :, pr * P:(pr + 1) * P],
                                  wk1[:, :, pr * P:(pr + 1) * P])
            bi_sb = p1.tile([P, NF], FP32, tag="bi")
            nc.sync.dma_start(bi_sb[:], bi[:])
            bo_sb = p1.tile([P, DN], FP32, tag="bo")
            bo_ap = bo[:]
            nc.sync.dma_start(
                bo_sb[:],
                bass.AP(tensor=bo_ap.tensor, offset=bo_ap.offset,
                        ap=[[0, P]] + list(bo_ap.ap[1:])))

            nsub = max(1, DN // 512)
            sub = DN // nsub

            def sub_norm(x_sb):
                stats = small.tile([P, nsub, 6], FP32, tag="stats")
                for i in range(nsub):
                    nc.vector.bn_stats(
                        out=stats[:, i, :], in_=x_sb[:, i * sub:(i + 1) * sub])
                mv = small.tile([P, 2], FP32, tag="mv")
                nc.vector.bn_aggr(out=mv[:], in_=stats[:])
                std = small.tile([P, 1], FP32, tag="std")
                nc.scalar.activation(
                    out=std[:], in_=mv[:, 1:2],
                    func=mybir.ActivationFunctionType.Sqrt,
                    scale=float(DN) / float(DN - 1))
                msum = small.tile([P, 1], FP32, tag="msum")
                nc.vector.tensor_add(out=msum[:], in0=mv[:, 0:1], in1=std[:])
                nc.vector.tensor_scalar_sub(out=x_sb[:], in0=x_sb[:],
                                            scalar1=msum[:])

            # SBUF-resident out1 and ff (true scale)
            out1_sb = p1.tile([P, HT, DN], BF16, tag="out1")
            ff_sb = p1.tile([P, HT, DN], BF16, tag="ff")

            def tail1_tile(j):
                # out1 rows j: sub_norm(m1 + y) -> out1_sb
                t = rows.tile([P, DN], FP32, tag="rows", name=f"t1_{j}")
                nc.sync.dma_start(t[:], ynat[j * P:(j + 1) * P, :])
                tb = rows.tile([P, DN], BF16, tag="rowsb", name=f"t1b_{j}")
                nc.sync.dma_start(
                    tb[:], bnc_out[0][j // 2][(j % 2) * P:(j % 2) * P + P, :])
                nc.gpsimd.tensor_add(out=t[:], in0=t[:], in1=tb[:])
                sub_norm(t)
                nc.vector.tensor_copy(out=out1_sb[:, j, :], in_=t[:])

            def tail23_tile(j):
                # out2 = sub_norm(out1 + m2); out = sub_norm(ff + out2 + bo)
                m2b = rows.tile([P, DN], BF16, tag="rowsb", name=f"m2b_{j}")
                nc.sync.dma_start(m2b[:], bnc_out[1][j][:])
                o2 = rows.tile([P, DN], FP32, tag="rows", name=f"o2_{j}")
                nc.vector.tensor_add(out=o2[:], in0=out1_sb[:, j, :], in1=m2b[:])
                sub_norm(o2)
                nc.vector.tensor_add(out=o2[:], in0=o2[:], in1=ff_sb[:, j, :])
                sub_norm(o2)
                nc.sync.dma_start(out[j * P:(j + 1) * P, :], o2[:])

            def reduce_chunk2(m, k):
                nc.gpsimd.collective_compute(
                    "ReduceScatter",
                    mybir.AluOpType.add,
                    replica_groups=[[2 * g, 2 * g + 1]
                                    for g in range(n_cores // 2)],
                    ins=[bnc_in[m][k].opt()],
                    outs=[bnc_out[m][k].opt()])

            def reduce_chunk(m, k):
                nc.gpsimd.collective_compute(
                    "ReduceScatter",
                    mybir.AluOpType.add,
                    replica_groups=[[2 * g, 2 * g + 1]
                                    for g in range(n_cores // 2)],
                    ins=[bnc_in[m][k].opt()],
                    outs=[bnc_out[m][k].opt()])

            # ---------------- building blocks
            def load_w(tag, src, shape, dt=FP8):
                t = p1.tile(shape, dt, tag=tag)
                nc.sync.dma_start(t[:], src[:])
                return t

            def qk_unit(dst, w_sb, kvT_sb, pair, th):
                tsl = slice(th * TH, (th + 1) * TH)
                pq = psum.tile([P, TH], FP32, tag="mm")
                for c in range(ND // 2):
                    nc.tensor.matmul(
                        pq[:], lhsT=w_sb[:, 2 * c:2 * c + 2,
                                         pair * P:(pair + 1) * P],
                        rhs=kvT_sb[:, 2 * c:2 * c + 2, tsl],
                        start=(c == 0), stop=(c == ND // 2 - 1),
                        perf_mode=DR)
                nc.vector.tensor_copy(out=dst[:, pair, tsl], in_=pq[:])

            def v_unit(wv_all, wv_sb, kvT_sb, st):
                pv = psum.tile([P, cfg.HV], FP32, tag="mm")
                for c in range(ND // 2):
                    nc.tensor.matmul(
                        pv[:], lhsT=kvT_sb[:, 2 * c:2 * c + 2,
                                           st * P:(st + 1) * P],
                        rhs=wv_sb[:, 2 * c:2 * c + 2, :],
                        start=(c == 0), stop=(c == ND // 2 - 1),
                        perf_mode=DR)
                nc.vector.tensor_copy(out=wv_all[:, st, :], in_=pv[:])

            def head_loop(mi, wqt_sb, wkt_sb, wv_all, pt_sb, units):
                """scores/exp of head h interleaved with attnV of head h-1.

                `units` is a list of (fn, args) filler closures; one is issued
                after each attnV pair-step so the in-order PE stream always
                has independent work queued while scores wait on the scalar
                exp pipeline (keeps the PE p-state at 2.4GHz).
                """
                uq = list(units)
                nslots = 8 * (NT // 2 + 1)
                state = {}

                def pop_units(slot):
                    want = len(units) * (slot + 1) // nslots
                    done = len(units) - len(uq)
                    for _ in range(want - done):
                        fn, args = uq.pop(0)
                        fn(*args)
                exp_scale = EXP_SCALE[mi - 1]
                att_m = ATT_M[mi - 1]

                def partial_prelude(h):
                    exps, denom = state.pop(h)
                    rden = small.tile([P, NT], FP32, tag="rden")
                    nc.vector.reciprocal(out=rden[:], in_=denom[:])
                    nc.vector.tensor_scalar_mul(out=rden[:], in0=rden[:],
                                                scalar1=att_m)
                    wvp = wvpp.tile([P, NT, DV], FP8, tag="wvp")
                    for st in range(NT):
                        nc.vector.tensor_scalar_mul(
                            out=wvp[:, st, :],
                            in0=wv_all[:, st, 64 * h:64 * h + 64],
                            scalar1=rden[:, st:st + 1])
                    pa = ppp.tile([64, TH], FP32, tag="pa", name=f"pa{mi}_{h}")
                    pb = ppp.tile([64, TH], FP32, tag="pb", name=f"pb{mi}_{h}")
                    return exps, wvp, pa, pb

                def partial_step(ctx_p, c):
                    exps, wvp, pa, pb = ctx_p
                    e = exps[c]
                    nc.tensor.matmul(
                        pa[:], lhsT=wvp[:, 2 * c:2 * c + 2, :],
                        rhs=e[:, :, 0:TH],
                        start=(c == 0), stop=(c == NT // 2 - 1),
                        perf_mode=DR, skip_group_check=True)
                    nc.tensor.matmul(
                        pb[:], lhsT=wvp[:, 2 * c:2 * c + 2, :],
                        rhs=e[:, :, TH:Tq],
                        start=(c == 0), stop=(c == NT // 2 - 1),
                        perf_mode=DR, skip_group_check=True)

                def partial_evict(ctx_p, h):
                    _, _, pa, pb = ctx_p
                    pair, j = h // 2, h % 2
                    lo, hi = 64 * j, 64 * j + 64
                    nc.vector.tensor_scalar_mul(
                        out=pt_sb[lo:hi, pair, 0:TH], in0=pa[:],
                        scalar1=PT_EVICT)
                    nc.vector.tensor_scalar_mul(
                        out=pt_sb[lo:hi, pair, TH:Tq], in0=pb[:],
                        scalar1=PT_EVICT)

                for h in range(8):
                    pair, j = h // 2, h % 2
                    lo, hi = 64 * j, 64 * j + 64
                    ctx_p = partial_prelude(h - 1) if h > 0 else None
                    denom = small.tile([P, NT], FP32, tag="denom")
                    exps = []
                    for c in range(NT // 2):
                        e = expp.tile([P, 2, Tq], FP8, tag="exp",
                                      name=f"exp{mi}_{h}_{c}")
                        for jj in range(2):
                            st = 2 * c + jj
                            ps = psc.tile([P, Tq], FP32, tag="sc")
                            for th in range(NTH):
                                nc.tensor.matmul(
                                    ps[:, th * TH:(th + 1) * TH],
                                    lhsT=wkt_sb[lo:hi, pair,
                                                st * P:(st + 1) * P],
                                    rhs=wqt_sb[lo:hi, pair,
                                               th * TH:(th + 1) * TH],
                                    start=True, stop=True)
                            nc.scalar.activation(
                                out=e[:, jj, :], in_=ps[:],
                                func=mybir.ActivationFunctionType.Exp,
                                scale=exp_scale,
                                accum_out=denom[:, st:st + 1])
                        exps.append(e)
                        if ctx_p is not None:
                            partial_step(ctx_p, c)
                        pop_units(h * (NT // 2 + 1) + c)
                    if ctx_p is not None:
                        partial_evict(ctx_p, h - 1)
                    state[h] = (exps, denom)
                    pop_units(h * (NT // 2 + 1) + NT // 2)
                ctx_p = partial_prelude(7)
                for c in range(NT // 2):
                    partial_step(ctx_p, c)
                partial_evict(ctx_p, 7)
                while uq:
                    fn, args = uq.pop(0)
                    fn(*args)

            def wo_tile(wo_sb, pt_sb, m, wo_evict, k, half):
                tt = k + HT * half
                mo = rows.tile([P, DN], BF16, tag="rowsb", name=f"mo{m}_{tt}")
                for dh in range(NDH):
                    po = psum.tile([P, TH], FP32, tag="mm")
                    for i2 in range(2):
                        nc.tensor.matmul(
                            po[:],
                            lhsT=pt_sb[:, 2 * i2:2 * i2 + 2,
                                       tt * P:(tt + 1) * P],
                            rhs=wo_sb[:, 2 * i2:2 * i2 + 2,
                                      dh * TH:(dh + 1) * TH],
                            start=(i2 == 0), stop=(i2 == 1),
                            perf_mode=DR)
                    nc.scalar.activation(
                        out=mo[:, dh * TH:(dh + 1) * TH], in_=po[:],
                        func=mybir.ActivationFunctionType.Copy,
                        scale=wo_evict)
                tt2 = k + HT * half
                if m == 0:
                    kk, row = tt2 % 4 // 2, (tt2 % 2) * P + (tt2 // 4) * 2 * P
                    nc.sync.dma_start(bnc_in[0][kk][row:row + P, :], mo[:])
                else:
                    nc.sync.dma_start(
                        bnc_in[1][k][half * P:half * P + P, :], mo[:])

            def h1_chunk(h1_sb, yTo_sb, fc):
                wi_c = wic.tile([P, ND, P], BF16, tag="wic", name=f"wic_{fc}")
                nc.sync.dma_start(wi_c[:], wi[fc])
                ph = psum.tile([P, TH], FP32, tag="mm")
                for dc in range(ND):
                    nc.tensor.matmul(
                        ph[:], lhsT=wi_c[:, dc, :],
                        rhs=yTo_sb[:, dc, :],
                        start=(dc == 0), stop=(dc == ND - 1))
                nc.vector.tensor_scalar(
                    out=h1_sb[:, fc, :], in0=ph[:],
                    scalar1=bi_sb[:, fc:fc + 1], scalar2=0.0,
                    op0=mybir.AluOpType.add, op1=mybir.AluOpType.max)

            def ffp_quarter(h1_sb, wog_sb, g, tl, dh):
                # += h1[:, g] @ wot[g] for own-row tile tl, d-half dh
                pf = psum.tile([P, TH], FP32, tag="mm")
                for i in range(GF):
                    nc.tensor.matmul(
                        pf[:], lhsT=h1_sb[:, g * GF + i, tl * P:(tl + 1) * P],
                        rhs=wog_sb[:, i, dh * TH:(dh + 1) * TH],
                        start=(i == 0), stop=(i == GF - 1))
                dst = ff_sb[:, tl, dh * TH:(dh + 1) * TH]
                if g == 0:
                    nc.vector.tensor_copy(out=dst, in_=pf[:])
                else:
                    nc.vector.tensor_add(out=dst, in0=dst, in1=pf[:])

            # ---------------- P0: mha1 Q/K then V projections
            wv1_sb = load_w("wv1", wv1, [P, ND, cfg.HV])
            wqt1 = qkt.tile([P, 4, Tq], BF16, tag="wqt", name="wqt1")
            wkt1 = qkt.tile([P, 4, Tq], BF16, tag="wkt", name="wkt1")
            for pair in range(4):
                for th in range(NTH):
                    qk_unit(wqt1, wq1_sb, yT_sb, pair, th)
                    qk_unit(wkt1, wk1_sb, yT_sb, pair, th)
            wv_all1 = p1.tile([P, NT, cfg.HV], BF16, tag="wv_all1")
            for st in range(NT):
                v_unit(wv_all1, wv1_sb, yT_sb, st)
            # DMAs needed from P1 onward (run behind P0/P1 compute)
            yTo_sb = p1.tile([P, ND, TH], BF16, tag="yTo")
            nc.sync.dma_start(yTo_sb[:], yTo[:])
            xT_sb = xw.tile([P, ND, Tq], FP8, tag="big", name="xT_sb")
            nc.sync.dma_start(xT_sb[:], xT[:])
            wq2_sb = load_w("wq2", wq2, [P, ND, cfg.HK])
            wk2_sb = load_w("wk2", wk2, [P, ND, cfg.HK])
            wv2_sb = load_w("wv2", wv2, [P, ND, cfg.HV])
            wo1_sb = load_w("wo1", wo1, [P, 4, DN])

            # ---------------- P1: mha1 heads + mha2 projections + h1 chunks
            pt1 = ptp.tile([P, 4, Tq], FP8, tag="pt", name="pt1")
            wqt2 = qkt.tile([P, 4, Tq], BF16, tag="wqt", name="wqt2")
            wkt2 = qkt.tile([P, 4, Tq], BF16, tag="wkt", name="wkt2")
            wv_all2 = p1.tile([P, NT, cfg.HV], BF16, tag="wv_all2")
            h1_sb = p1.tile([P, NF, TH], BF16, tag="h1")

            p1_units = []
            for pair in range(4):
                for th in range(NTH):
                    # Q2 projects from y; K2/V2 from x (reference: mha(y,x,x))
                    p1_units.append((qk_unit, (wqt2, wq2_sb, yT_sb, pair, th)))
                    p1_units.append((qk_unit, (wkt2, wk2_sb, xT_sb, pair, th)))
                p1_units.append((v_unit, (wv_all2, wv2_sb, xT_sb, 2 * pair)))
                p1_units.append((v_unit, (wv_all2, wv2_sb, xT_sb, 2 * pair + 1)))
                p1_units.append((h1_chunk, (h1_sb, yTo_sb, 2 * pair)))
                p1_units.append((h1_chunk, (h1_sb, yTo_sb, 2 * pair + 1)))

            head_loop(1, wqt1, wkt1, wv_all1, pt1, p1_units)

            # ---------------- P2: Wo1 -> 2-chunk RS0 ; FFN-in fc=8..15
            for k2 in range(2):
                for tt in (2 * k2, 2 * k2 + 1, 2 * k2 + 4, 2 * k2 + 5):
                    wo_tile(wo1_sb, pt1, 0, WO_EVICT[0], tt % 4, tt // 4)
                reduce_chunk2(0, k2)
                for fc in range(8 + 8 * k2, 16 + 8 * k2):
                    h1_chunk(h1_sb, yTo_sb, fc)
            # wot groups 0/1 for P3 (slot A frees when xT dies end-P1)
            wog = [None] * NG
            for g in range(2):
                wog[g] = xw.tile([P, GF, DN], BF16, tag="big", name=f"wog{g}")
                nc.sync.dma_start(wog[g][:], wot[:, g * GF:(g + 1) * GF, :])
            wo2_sb = load_w("wo1", wo2, [P, 4, DN])

            # ---------------- P3: mha2 heads + FFN-out groups 0-1 + tail1
            pt2 = ptp.tile([P, 4, Tq], FP8, tag="pt", name="pt2")

            def load_wog2():
                wog[2] = xw.tile([P, GF, DN], BF16, tag="big", name="wog2")
                nc.sync.dma_start(wog[2][:], wot[:, 2 * GF:3 * GF, :])

            p3_units = []
            for tl in range(HT):
                p3_units.append((ffp_quarter, (h1_sb, wog[0], 0, tl, 0)))
                p3_units.append((ffp_quarter, (h1_sb, wog[0], 0, tl, 1)))
            p3_units.append((load_wog2, ()))
            for tl in range(HT):
                p3_units.append((tail1_tile, (tl,)))
                p3_units.append((ffp_quarter, (h1_sb, wog[1], 1, tl, 0)))
                p3_units.append((ffp_quarter, (h1_sb, wog[1], 1, tl, 1)))

            head_loop(2, wqt2, wkt2, wv_all2, pt2, p3_units)

            # ---------------- P4: Wo2 -> chunked RS1 ; groups 2-3 ; tails
            wog[3] = xw.tile([P, GF, DN], BF16, tag="big", name="wog3")
            nc.sync.dma_start(wog[3][:], wot[:, 3 * GF:4 * GF, :])
            for k in range(HT):
                wo_tile(wo2_sb, pt2, 1, WO_EVICT[1], k, 0)
                wo_tile(wo2_sb, pt2, 1, WO_EVICT[1], k, 1)
                reduce_chunk(1, k)
                for fc in range(24 + 2 * k, 26 + 2 * k):
                    h1_chunk(h1_sb, yTo_sb, fc)
            for j in range(HT):
                ffp_quarter(h1_sb, wog[2], 2, j, 0)
                ffp_quarter(h1_sb, wog[2], 2, j, 1)
                ffp_quarter(h1_sb, wog[3], 3, j, 0)
                ffp_quarter(h1_sb, wog[3], 3, j, 1)
                nc.gpsimd.tensor_add(out=ff_sb[:, j, :],
                                     in0=ff_sb[:, j, :], in1=bo_sb[:])
                tail23_tile(j)

    if compile:
        nc.compile()
    return nc


# ---------------------------------------------------------------- host side

def _pow2_scale(a, target=112.0):
    m = float(np.abs(a).max())
    if m == 0.0:
        return 1.0
    return float(2.0 ** math.floor(math.log2(target / m)))


def _q8(a, s):
    return np.clip(np.asarray(a, np.float32) * s, -224.0, 224.0).astype(NPF8)


def compute_scales(Wq1, Wk1, Wv1, Wo1, Wq2, Wk2, Wv2, Wo2, W_in, W_out):
    inv = np.float32(1.0 / np.sqrt(np.float32(DK)))
    return dict(
        wq1=_pow2_scale(Wq1 * inv), wk1=_pow2_scale(Wk1),
        wv1=_pow2_scale(Wv1), wo1=_pow2_scale(Wo1),
        wq2=_pow2_scale(Wq2 * inv), wk2=_pow2_scale(Wk2),
        wv2=_pow2_scale(Wv2), wo2=_pow2_scale(Wo2),
    )


def pack_inputs(cfg, scales, x, y, Wq1, Wk1, Wv1, Wo1, Wq2, Wk2, Wv2, Wo2,
                W_in, b_in, W_out, b_out):
    Tq, DN, ND, NF = cfg.T, cfg.D, cfg.ND, cfg.NF
    NH = H // 2
    TH = cfg.TH

    def tr8(a):
        return _q8(np.ascontiguousarray(
            a.T.reshape(ND, P, Tq).transpose(1, 0, 2)), S_Y)

    def trbf(a):  # [TH, D] -> [P, ND, TH] bf16 true scale
        return np.ascontiguousarray(
            a.T.reshape(ND, P, TH).transpose(1, 0, 2)).astype(NPBF16)

    def qk_pack(W, h0, s):
        Wh = W[h0:h0 + NH]
        Wp = Wh.reshape(NH // 2, 2, DN, DK).transpose(2, 0, 1, 3)
        Wp = Wp.reshape(DN, NH * DK)
        return _q8(np.ascontiguousarray(
            Wp.reshape(ND, P, NH * DK).transpose(1, 0, 2)), s)

    def v_pack(W, h0, s):
        Wh = W[h0:h0 + NH].transpose(1, 0, 2).reshape(DN, NH * DV)
        return _q8(np.ascontiguousarray(
            Wh.reshape(ND, P, NH * DV).transpose(1, 0, 2)), s)

    def wo_pack(Wo, h, s):
        Ws = Wo[NH * DV * h:NH * DV * h + NH * DV]
        return _q8(np.ascontiguousarray(
            Ws.reshape(4, P, DN).transpose(1, 0, 2)), s)

    def wi_pack(W_in):  # full FF -> [NF, P, ND, P] bf16 chunked lhsT[d, f]
        A = np.asarray(W_in, np.float32).T.reshape(ND, P, NF, P)
        return np.ascontiguousarray(A.transpose(2, 1, 0, 3)).astype(NPBF16)

    def wot_pack(W_out):
        Ws = np.asarray(W_out, np.float32).T      # [FF, D]
        return np.ascontiguousarray(
            Ws.reshape(NF, P, DN).transpose(1, 0, 2)).astype(NPBF16)

    inv = np.float32(1.0 / np.sqrt(np.float32(DK)))
    wi_b = wi_pack(W_in)
    wot_b = wot_pack(W_out)
    bi_b = np.ascontiguousarray(
        np.asarray(b_in, np.float32).reshape(NF, P).T)
    in_maps = []
    for c in range(2 * x.shape[0]):
        b, h = c // 2, c % 2
        h0 = NH * h
        in_maps.append(dict(
            yT=tr8(y[b]),
            yTo=trbf(y[b, h * TH:(h + 1) * TH]),
            xT=tr8(x[b]),
            ynat=np.ascontiguousarray(y[b, h * TH:(h + 1) * TH]).astype(np.float32),
            wq1=qk_pack(Wq1 * inv, h0, scales["wq1"]),
            wk1=qk_pack(Wk1, h0, scales["wk1"]),
            wv1=v_pack(Wv1, h0, scales["wv1"]),
            wo1=wo_pack(Wo1, h, scales["wo1"]),
            wq2=qk_pack(Wq2 * inv, h0, scales["wq2"]),
            wk2=qk_pack(Wk2, h0, scales["wk2"]),
            wv2=v_pack(Wv2, h0, scales["wv2"]),
            wo2=wo_pack(Wo2, h, scales["wo2"]),
            wi=wi_b, wot=wot_b, bi=bi_b,
            bo=np.asarray(b_out).reshape(1, DN).astype(np.float32),
        ))
    return in_maps


_PROG_CACHE = {}


def kernel(**inputs) -> np.ndarray:
    cfg = Cfg()
    inputs = {k: np.asarray(v, np.float32) for k, v in inputs.items()}
    scales = compute_scales(
        inputs["Wq1"], inputs["Wk1"], inputs["Wv1"], inputs["Wo1"],
        inputs["Wq2"], inputs["Wk2"], inputs["Wv2"], inputs["Wo2"],
        inputs["W_in"], inputs["W_out"])
    key = tuple(sorted(scales.items()))
    if key not in _PROG_CACHE:
        _PROG_CACHE[key] = build_program(cfg, scales)
    nc = _PROG_CACHE[key]
    in_maps = pack_inputs(cfg, scales, **inputs)
    res = run_bass_kernel_spmd(nc, in_maps, core_ids=list(range(8)))
    TH = cfg.TH
    out = np.empty((B, T, D), np.float32)
    for c in range(8):
        b, h = c // 2, c % 2
        out[b, h * TH:(h + 1) * TH] = res.results[c]["out"]
    return out


# revision 13
# speedup vs baseline: 1.1353x; 1.1353x over previous
"""Trainium2 Bass kernel for nn_DecoderStack — v4.

Sharding over 8 NeuronCores: core c -> batch b=c//2, half h=c%2.
MHA1/MHA2: head-split (8 heads/core), pair ReduceScatter after Wo (chunked
4x[256,D] bf16, pipelined with tails).  FFN: t-split (full FF=4096 over the
core's own 512 rows), entirely bf16 — no FFN collective.

fp8e4 DoubleRow matmuls: QKV projections, attnV (V pre-scaled by 1/denom),
Wo.  bf16: scores (K=64) and the whole FFN (fp8 there costs too much
accuracy).  Dequant scales are exact powers of two folded into the Exp
activation scale and PSUM-eviction multiplies.

Engine split: PE matmuls; scalar exp (the head-loop pacer) + sqrt; vector
qk/v/pt/wvp evictions + tails; pool (gpsimd) wo/h1/ffp evictions.

Schedule (PE kept dense to hold the 2.4GHz p-state):
  P0: mha1 Q/K/V projection units
  P1: mha1 head loop; filler: 3 mha2-projection units + 1 FFN-in chunk/head
  P2: Wo1 tiles -> chunked RS0 ; FFN-in chunks fc=8..31
  P3: mha2 head loop; filler: FFN-out quarters (wot groups 0-1) + tail1
  P4: Wo2 tiles (k,k+4) -> RS1 chunk k ; FFN-out groups 2-3 between chunks;
      fused tail2+tail3 per chunk
"""

import sys

for _p in ("/opt/trn_rl_repo", "/root/.axon_site"):
    if _p not in sys.path:
        sys.path.insert(0, _p)

import contextlib
import math

import numpy as np

import concourse.bass as bass
import concourse.bacc as bacc
import concourse.tile as tile
from concourse import mybir
from concourse.bass_utils import run_bass_kernel_spmd

B, T, D, H, DK, DV, FF = 4, 1024, 1024, 16, 64, 64, 4096
P = 128
FP32 = mybir.dt.float32
BF16 = mybir.dt.bfloat16
FP8 = mybir.dt.float8e4
NPBF16 = mybir.dt.np(BF16)
NPF8 = mybir.dt.np(FP8)
DR = mybir.MatmulPerfMode.DoubleRow

# quantization scales (exact powers of two)
S_Y = 32.0          # y, x inputs (sigma ~1)
S_ATT = 4096.0      # attn-partial psum scale (baked into wvp)
S_PT = 32.0         # pt fp8 scale


class Cfg:
    def __init__(self, T_=T, D_=D, FF_=FF):
        self.T = T_
        self.D = D_
        self.FF = FF_
        self.NT = T_ // P          # 8 t/s tiles
        self.ND = D_ // P          # 8 d chunks
        self.HT = T_ // 2 // P     # 4 own-row tiles
        self.NF = FF_ // P         # 32 f chunks (full FF)
        self.NG = 4                # wot fc-groups
        self.GF = self.NF // self.NG   # 8 fc per group
        self.HK = 8 * DK           # 512
        self.HV = 8 * DV           # 512
        self.TH = T_ // 2          # 512 own rows


def build_program(cfg: Cfg, scales: dict, n_cores: int = 8, compile: bool = True):
    nc = bacc.Bacc("TRN2", target_bir_lowering=False, debug=False,
                   num_devices=n_cores)
    NT, ND, NF, HT, TH = cfg.NT, cfg.ND, cfg.NF, cfg.HT, cfg.TH
    NG, GF = cfg.NG, cfg.GF
    Tq, DN = cfg.T, cfg.D
    NTH = 2
    NDH = DN // TH

    EXP_SCALE = (1.0 / (S_Y * S_Y * scales["wq1"] * scales["wk1"]),
                 1.0 / (S_Y * S_Y * scales["wq2"] * scales["wk2"]))
    ATT_M = (S_ATT / (S_Y * scales["wv1"]), S_ATT / (S_Y * scales["wv2"]))
    PT_EVICT = S_PT / S_ATT
    WO_EVICT = (1.0 / (S_PT * scales["wo1"]), 1.0 / (S_PT * scales["wo2"]))

    def dram_in(name, shape, dt=FP8):
        return nc.dram_tensor(name, shape, dt, kind="ExternalInput")

    yT = dram_in("yT", [P, ND, Tq])
    yTo = dram_in("yTo", [P, ND, TH], BF16)    # own t-half, true scale
    xT = dram_in("xT", [P, ND, Tq])
    ynat = dram_in("ynat", [TH, DN], FP32)
    wq1 = dram_in("wq1", [P, ND, cfg.HK])
    wk1 = dram_in("wk1", [P, ND, cfg.HK])
    wv1 = dram_in("wv1", [P, ND, cfg.HV])
    wo1 = dram_in("wo1", [P, 4, DN])
    wq2 = dram_in("wq2", [P, ND, cfg.HK])
    wk2 = dram_in("wk2", [P, ND, cfg.HK])
    wv2 = dram_in("wv2", [P, ND, cfg.HV])
    wo2 = dram_in("wo2", [P, 4, DN])
    wi = dram_in("wi", [NF, P, ND, P], BF16)   # W_in chunks, lhsT[d, f]
    wot = dram_in("wot", [P, NF, DN], BF16)
    bi = dram_in("bi", [P, NF], FP32)
    bo = dram_in("bo", [1, DN], FP32)
    out = nc.dram_tensor("out", [TH, DN], FP32, kind="ExternalOutput")

    with tile.TileContext(nc) as tc:
        with contextlib.ExitStack() as ctx:
            p1 = ctx.enter_context(tc.tile_pool(name="p1", bufs=1))
            xw = ctx.enter_context(tc.tile_pool(name="xw", bufs=2))
            qkt = ctx.enter_context(tc.tile_pool(name="qkt", bufs=2))
            ptp = ctx.enter_context(tc.tile_pool(name="ptp", bufs=1))
            expp = ctx.enter_context(tc.tile_pool(name="expp", bufs=4))
            wvpp = ctx.enter_context(tc.tile_pool(name="wvpp", bufs=2))
            wic = ctx.enter_context(tc.tile_pool(name="wic", bufs=2))
            rows = ctx.enter_context(tc.tile_pool(name="rows", bufs=2))
            small = ctx.enter_context(tc.tile_pool(name="small", bufs=2))
            psum = ctx.enter_context(tc.tile_pool(name="psum", bufs=2, space="PSUM"))
            psc = ctx.enter_context(tc.tile_pool(name="psc", bufs=2, space="PSUM"))
            ppp = ctx.enter_context(tc.tile_pool(name="ppp", bufs=1, space="PSUM"))
            dram = ctx.enter_context(tc.tile_pool(name="dram", bufs=1, space="DRAM"))

            # chunked RS staging: RS0: 2x in [512, DN]; RS1: 4x in [256, DN]
            bnc_in = [[dram.tile([4 * P, DN], BF16, tag=f"bi0{k}",
                                 name=f"bi0{k}") for k in range(2)],
                      [dram.tile([2 * P, DN], BF16, tag=f"bi1{k}",
                                 name=f"bi1{k}") for k in range(HT)]]
            bnc_out = [[dram.tile([2 * P, DN], BF16, tag=f"bo0{k}",
                                  name=f"bo0{k}") for k in range(2)],
                       [dram.tile([P, DN], BF16, tag=f"bo1{k}",
                                  name=f"bo1{k}") for k in range(HT)]]

            # ---------------- persistent loads (order matters for startup)
            yT_sb = p1.tile([P, ND, Tq], FP8, tag="yT")
            nc.sync.dma_start(yT_sb[:], yT[:])
            wq1_sb = p1.tile([P, ND, cfg.HK], FP8, tag="wq1")
            wk1_sb = p1.tile([P, ND, cfg.HK], FP8, tag="wk1")
            for pr in range(4):
                nc.sync.dma_start(wq1_sb[:, :, pr * P:(pr + 1) * P],
                                  wq1[:, :, pr * P:(pr + 1) * P])
                nc.sync.dma_start(wk1_sb[:, :, pr * P:(pr + 1) * P],
                                  wk1[:, :, pr * P:(pr + 1) * P])
            bi_sb = p1.tile([P, NF], FP32, tag="bi")
            nc.sync.dma_start(bi_sb[:], bi[:])
            bo_sb = p1.tile([P, DN], FP32, tag="bo")
            bo_ap = bo[:]
            nc.sync.dma_start(
                bo_sb[:],
                bass.AP(tensor=bo_ap.tensor, offset=bo_ap.offset,
                        ap=[[0, P]] + list(bo_ap.ap[1:])))

            nsub = max(1, DN // 512)
            sub = DN // nsub

            def sub_norm(x_sb):
                stats = small.tile([P, nsub, 6], FP32, tag="stats")
                for i in range(nsub):
                    nc.vector.bn_stats(
                        out=stats[:, i, :], in_=x_sb[:, i * sub:(i + 1) * sub])
                mv = small.tile([P, 2], FP32, tag="mv")
                nc.vector.bn_aggr(out=mv[:], in_=stats[:])
                std = small.tile([P, 1], FP32, tag="std")
                nc.scalar.activation(
                    out=std[:], in_=mv[:, 1:2],
                    func=mybir.ActivationFunctionType.Sqrt,
                    scale=float(DN) / float(DN - 1))
                msum = small.tile([P, 1], FP32, tag="msum")
                nc.vector.tensor_add(out=msum[:], in0=mv[:, 0:1], in1=std[:])
                nc.vector.tensor_scalar_sub(out=x_sb[:], in0=x_sb[:],
                                            scalar1=msum[:])

            # SBUF-resident out1 and ff (true scale)
            out1_sb = p1.tile([P, HT, DN], BF16, tag="out1")
            ff_sb = p1.tile([P, HT, DN], BF16, tag="ff")

            def tail1_tile(j):
                # out1 rows j: sub_norm(m1 + y) -> out1_sb
                t = rows.tile([P, DN], FP32, tag="rows", name=f"t1_{j}")
                nc.sync.dma_start(t[:], ynat[j * P:(j + 1) * P, :])
                tb = rows.tile([P, DN], BF16, tag="rowsb", name=f"t1b_{j}")
                nc.sync.dma_start(
                    tb[:], bnc_out[0][j // 2][(j % 2) * P:(j % 2) * P + P, :])
                nc.gpsimd.tensor_add(out=t[:], in0=t[:], in1=tb[:])
                sub_norm(t)
                nc.vector.tensor_copy(out=out1_sb[:, j, :], in_=t[:])

            def tail23_tile(j):
                # out2 = sub_norm(out1 + m2); out = sub_norm(ff + out2 + bo)
                m2b = rows.tile([P, DN], BF16, tag="rowsb", name=f"m2b_{j}")
                nc.sync.dma_start(m2b[:], bnc_out[1][j][:])
                o2 = rows.tile([P, DN], FP32, tag="rows", name=f"o2_{j}")
                nc.vector.tensor_add(out=o2[:], in0=out1_sb[:, j, :], in1=m2b[:])
                sub_norm(o2)
                nc.vector.tensor_add(out=o2[:], in0=o2[:], in1=ff_sb[:, j, :])
                sub_norm(o2)
                nc.sync.dma_start(out[j * P:(j + 1) * P, :], o2[:])

            def reduce_chunk2(m, k):
                nc.gpsimd.collective_compute(
                    "ReduceScatter",
                    mybir.AluOpType.add,
                    replica_groups=[[2 * g, 2 * g + 1]
                                    for g in range(n_cores // 2)],
                    ins=[bnc_in[m][k].opt()],
                    outs=[bnc_out[m][k].opt()])

            def reduce_chunk(m, k):
                nc.gpsimd.collective_compute(
                    "ReduceScatter",
                    mybir.AluOpType.add,
                    replica_groups=[[2 * g, 2 * g + 1]
                                    for g in range(n_cores // 2)],
                    ins=[bnc_in[m][k].opt()],
                    outs=[bnc_out[m][k].opt()])

            # ---------------- building blocks
            def load_w(tag, src, shape, dt=FP8):
                t = p1.tile(shape, dt, tag=tag)
                nc.sync.dma_start(t[:], src[:])
                return t

            def qk_unit(dst, w_sb, kvT_sb, pair, th):
                tsl = slice(th * TH, (th + 1) * TH)
                pq = psum.tile([P, TH], FP32, tag="mm")
                for c in range(ND // 2):
                    nc.tensor.matmul(
                        pq[:], lhsT=w_sb[:, 2 * c:2 * c + 2,
                                         pair * P:(pair + 1) * P],
                        rhs=kvT_sb[:, 2 * c:2 * c + 2, tsl],
                        start=(c == 0), stop=(c == ND // 2 - 1),
                        perf_mode=DR)
                nc.vector.tensor_copy(out=dst[:, pair, tsl], in_=pq[:])

            def v_unit(wv_all, wv_sb, kvT_sb, st):
                pv = psum.tile([P, cfg.HV], FP32, tag="mm")
                for c in range(ND // 2):
                    nc.tensor.matmul(
                        pv[:], lhsT=kvT_sb[:, 2 * c:2 * c + 2,
                                           st * P:(st + 1) * P],
                        rhs=wv_sb[:, 2 * c:2 * c + 2, :],
                        start=(c == 0), stop=(c == ND // 2 - 1),
                        perf_mode=DR)
                nc.vector.tensor_copy(out=wv_all[:, st, :], in_=pv[:])

            def head_loop(mi, wqt_sb, wkt_sb, wv_all, pt_sb, units):
                """scores/exp of head h interleaved with attnV of head h-1.

                `units` is a list of (fn, args) filler closures; one is issued
                after each attnV pair-step so the in-order PE stream always
                has independent work queued while scores wait on the scalar
                exp pipeline (keeps the PE p-state at 2.4GHz).
                """
                uq = list(units)
                nslots = 8 * (NT // 2 + 1)
                state = {}

                def pop_units(slot):
                    want = len(units) * (slot + 1) // nslots
                    done = len(units) - len(uq)
                    for _ in range(want - done):
                        fn, args = uq.pop(0)
                        fn(*args)
                exp_scale = EXP_SCALE[mi - 1]
                att_m = ATT_M[mi - 1]

                def partial_prelude(h):
                    exps, denom = state.pop(h)
                    rden = small.tile([P, NT], FP32, tag="rden")
                    nc.vector.reciprocal(out=rden[:], in_=denom[:])
                    nc.vector.tensor_scalar_mul(out=rden[:], in0=rden[:],
                                                scalar1=att_m)
                    wvp = wvpp.tile([P, NT, DV], FP8, tag="wvp")
                    for st in range(NT):
                        nc.vector.tensor_scalar_mul(
                            out=wvp[:, st, :],
                            in0=wv_all[:, st, 64 * h:64 * h + 64],
                            scalar1=rden[:, st:st + 1])
                    pa = ppp.tile([64, TH], FP32, tag="pa", name=f"pa{mi}_{h}")
                    pb = ppp.tile([64, TH], FP32, tag="pb", name=f"pb{mi}_{h}")
                    return exps, wvp, pa, pb

                def partial_step(ctx_p, c):
                    exps, wvp, pa, pb = ctx_p
                    e = exps[c]
                    nc.tensor.matmul(
                        pa[:], lhsT=wvp[:, 2 * c:2 * c + 2, :],
                        rhs=e[:, :, 0:TH],
                        start=(c == 0), stop=(c == NT // 2 - 1),
                        perf_mode=DR, skip_group_check=True)
                    nc.tensor.matmul(
                        pb[:], lhsT=wvp[:, 2 * c:2 * c + 2, :],
                        rhs=e[:, :, TH:Tq],
                        start=(c == 0), stop=(c == NT // 2 - 1),
                        perf_mode=DR, skip_group_check=True)

                def partial_evict(ctx_p, h):
                    _, _, pa, pb = ctx_p
                    pair, j = h // 2, h % 2
                    lo, hi = 64 * j, 64 * j + 64
                    nc.vector.tensor_scalar_mul(
                        out=pt_sb[lo:hi, pair, 0:TH], in0=pa[:],
                        scalar1=PT_EVICT)
                    nc.vector.tensor_scalar_mul(
                        out=pt_sb[lo:hi, pair, TH:Tq], in0=pb[:],
                        scalar1=PT_EVICT)

                for h in range(8):
                    pair, j = h // 2, h % 2
                    lo, hi = 64 * j, 64 * j + 64
                    ctx_p = partial_prelude(h - 1) if h > 0 else None
                    denom = small.tile([P, NT], FP32, tag="denom")
                    exps = []
                    for c in range(NT // 2):
                        e = expp.tile([P, 2, Tq], FP8, tag="exp",
                                      name=f"exp{mi}_{h}_{c}")
                        for jj in range(2):
                            st = 2 * c + jj
                            ps = psc.tile([P, Tq], FP32, tag="sc")
                            for th in range(NTH):
                                nc.tensor.matmul(
                                    ps[:, th * TH:(th + 1) * TH],
                                    lhsT=wkt_sb[lo:hi, pair,
                                                st * P:(st + 1) * P],
                                    rhs=wqt_sb[lo:hi, pair,
                                               th * TH:(th + 1) * TH],
                                    start=True, stop=True)
                            nc.scalar.activation(
                                out=e[:, jj, :], in_=ps[:],
                                func=mybir.ActivationFunctionType.Exp,
                                scale=exp_scale,
                                accum_out=denom[:, st:st + 1])
                        exps.append(e)
                        if ctx_p is not None:
                            partial_step(ctx_p, c)
                        pop_units(h * (NT // 2 + 1) + c)
                    if ctx_p is not None:
                        partial_evict(ctx_p, h - 1)
                    state[h] = (exps, denom)
                    pop_units(h * (NT // 2 + 1) + NT // 2)
                ctx_p = partial_prelude(7)
                for c in range(NT // 2):
                    partial_step(ctx_p, c)
                partial_evict(ctx_p, 7)
                while uq:
                    fn, args = uq.pop(0)
                    fn(*args)

            def wo_tile(wo_sb, pt_sb, m, wo_evict, k, half):
                tt = k + HT * half
                mo = rows.tile([P, DN], BF16, tag="rowsb", name=f"mo{m}_{tt}")
                for dh in range(NDH):
                    po = psum.tile([P, TH], FP32, tag="mm")
                    for i2 in range(2):
                        nc.tensor.matmul(
                            po[:],
                            lhsT=pt_sb[:, 2 * i2:2 * i2 + 2,
                                       tt * P:(tt + 1) * P],
                            rhs=wo_sb[:, 2 * i2:2 * i2 + 2,
                                      dh * TH:(dh + 1) * TH],
                            start=(i2 == 0), stop=(i2 == 1),
                            perf_mode=DR)
                    nc.scalar.activation(
                        out=mo[:, dh * TH:(dh + 1) * TH], in_=po[:],
                        func=mybir.ActivationFunctionType.Copy,
                        scale=wo_evict)
                tt2 = k + HT * half
                if m == 0:
                    kk, row = tt2 % 4 // 2, (tt2 % 2) * P + (tt2 // 4) * 2 * P
                    nc.sync.dma_start(bnc_in[0][kk][row:row + P, :], mo[:])
                else:
                    nc.sync.dma_start(
                        bnc_in[1][k][half * P:half * P + P, :], mo[:])

            def h1_chunk(h1_sb, yTo_sb, fc):
                wi_c = wic.tile([P, ND, P], BF16, tag="wic", name=f"wic_{fc}")
                nc.sync.dma_start(wi_c[:], wi[fc])
                ph = psum.tile([P, TH], FP32, tag="mm")
                for dc in range(ND):
                    nc.tensor.matmul(
                        ph[:], lhsT=wi_c[:, dc, :],
                        rhs=yTo_sb[:, dc, :],
                        start=(dc == 0), stop=(dc == ND - 1))
                nc.vector.tensor_scalar(
                    out=h1_sb[:, fc, :], in0=ph[:],
                    scalar1=bi_sb[:, fc:fc + 1], scalar2=0.0,
                    op0=mybir.AluOpType.add, op1=mybir.AluOpType.max)

            def ffp_quarter(h1_sb, wog_sb, g, tl, dh):
                # += h1[:, g] @ wot[g] for own-row tile tl, d-half dh
                pf = psum.tile([P, TH], FP32, tag="mm")
                for i in range(GF):
                    nc.tensor.matmul(
                        pf[:], lhsT=h1_sb[:, g * GF + i, tl * P:(tl + 1) * P],
                        rhs=wog_sb[:, i, dh * TH:(dh + 1) * TH],
                        start=(i == 0), stop=(i == GF - 1))
                dst = ff_sb[:, tl, dh * TH:(dh + 1) * TH]
                if g == 0:
                    nc.vector.tensor_copy(out=dst, in_=pf[:])
                else:
                    nc.vector.tensor_add(out=dst, in0=dst, in1=pf[:])

            # ---------------- P0: mha1 Q/K then V projections
            wv1_sb = load_w("wv1", wv1, [P, ND, cfg.HV])
            wqt1 = qkt.tile([P, 4, Tq], BF16, tag="wqt", name="wqt1")
            wkt1 = qkt.tile([P, 4, Tq], BF16, tag="wkt", name="wkt1")
            for pair in range(4):
                for th in range(NTH):
                    qk_unit(wqt1, wq1_sb, yT_sb, pair, th)
                    qk_unit(wkt1, wk1_sb, yT_sb, pair, th)
            wv_all1 = p1.tile([P, NT, cfg.HV], BF16, tag="wv_all1")
            for st in range(NT):
                v_unit(wv_all1, wv1_sb, yT_sb, st)
            # DMAs needed from P1 onward (run behind P0/P1 compute)
            yTo_sb = p1.tile([P, ND, TH], BF16, tag="yTo")
            nc.sync.dma_start(yTo_sb[:], yTo[:])
            xT_sb = xw.tile([P, ND, Tq], FP8, tag="big", name="xT_sb")
            nc.sync.dma_start(xT_sb[:], xT[:])
            wq2_sb = load_w("wq2", wq2, [P, ND, cfg.HK])
            wk2_sb = load_w("wk2", wk2, [P, ND, cfg.HK])
            wv2_sb = load_w("wv2", wv2, [P, ND, cfg.HV])
            wo1_sb = load_w("wo1", wo1, [P, 4, DN])

            # ---------------- P1: mha1 heads + mha2 projections + h1 chunks
            pt1 = ptp.tile([P, 4, Tq], FP8, tag="pt", name="pt1")
            wqt2 = qkt.tile([P, 4, Tq], BF16, tag="wqt", name="wqt2")
            wkt2 = qkt.tile([P, 4, Tq], BF16, tag="wkt", name="wkt2")
            wv_all2 = p1.tile([P, NT, cfg.HV], BF16, tag="wv_all2")
            h1_sb = p1.tile([P, NF, TH], BF16, tag="h1")

            p1_units = []
            for pair in range(4):
                for th in range(NTH):
                    # Q2 projects from y; K2/V2 from x (reference: mha(y,x,x))
                    p1_units.append((qk_unit, (wqt2, wq2_sb, yT_sb, pair, th)))
                    p1_units.append((qk_unit, (wkt2, wk2_sb, xT_sb, pair, th)))
                p1_units.append((v_unit, (wv_all2, wv2_sb, xT_sb, 2 * pair)))
                p1_units.append((v_unit, (wv_all2, wv2_sb, xT_sb, 2 * pair + 1)))
                p1_units.append((h1_chunk, (h1_sb, yTo_sb, 2 * pair)))
                p1_units.append((h1_chunk, (h1_sb, yTo_sb, 2 * pair + 1)))

            head_loop(1, wqt1, wkt1, wv_all1, pt1, p1_units)

            # ---------------- P2: Wo1 -> 2-chunk RS0 ; FFN-in fc=8..15
            for k2 in range(2):
                for tt in (2 * k2, 2 * k2 + 1, 2 * k2 + 4, 2 * k2 + 5):
                    wo_tile(wo1_sb, pt1, 0, WO_EVICT[0], tt % 4, tt // 4)
                reduce_chunk2(0, k2)
                for fc in range(8 + 4 * k2, 12 + 4 * k2):
                    h1_chunk(h1_sb, yTo_sb, fc)
            # wot groups 0/1 for P3 (slot A frees when xT dies end-P1)
            wog = [None] * NG
            for g in range(2):
                wog[g] = xw.tile([P, GF, DN], BF16, tag="big", name=f"wog{g}")
                nc.sync.dma_start(wog[g][:], wot[:, g * GF:(g + 1) * GF, :])
            wo2_sb = load_w("wo1", wo2, [P, 4, DN])

            # ---------------- P3: mha2 heads + FFN-out groups 0-1 + tail1
            pt2 = ptp.tile([P, 4, Tq], FP8, tag="pt", name="pt2")

            def load_wog2():
                wog[2] = xw.tile([P, GF, DN], BF16, tag="big", name="wog2")
                nc.sync.dma_start(wog[2][:], wot[:, 2 * GF:3 * GF, :])

            p3_units = []
            for tl in range(HT):
                p3_units.append((ffp_quarter, (h1_sb, wog[0], 0, tl, 0)))
                p3_units.append((ffp_quarter, (h1_sb, wog[0], 0, tl, 1)))
            p3_units.append((load_wog2, ()))
            for tl in range(HT):
                p3_units.append((tail1_tile, (tl,)))
                p3_units.append((ffp_quarter, (h1_sb, wog[1], 1, tl, 0)))
                p3_units.append((ffp_quarter, (h1_sb, wog[1], 1, tl, 1)))

            head_loop(2, wqt2, wkt2, wv_all2, pt2, p3_units)

            # ---------------- P4: Wo2 -> chunked RS1 ; groups 2-3 ; tails
            wog[3] = xw.tile([P, GF, DN], BF16, tag="big", name="wog3")
            nc.sync.dma_start(wog[3][:], wot[:, 3 * GF:4 * GF, :])
            for k in range(HT):
                wo_tile(wo2_sb, pt2, 1, WO_EVICT[1], k, 0)
                wo_tile(wo2_sb, pt2, 1, WO_EVICT[1], k, 1)
                reduce_chunk(1, k)
                for fc in range(16 + 2 * k, 18 + 2 * k):
                    h1_chunk(h1_sb, yTo_sb, fc)
            for k in range(HT):
                for fc in range(24 + 2 * k, 26 + 2 * k):
                    h1_chunk(h1_sb, yTo_sb, fc)
                ffp_quarter(h1_sb, wog[2], 2, k, 0)
                ffp_quarter(h1_sb, wog[2], 2, k, 1)
                nc.gpsimd.tensor_add(out=ff_sb[:, k, 0:TH],
                                     in0=ff_sb[:, k, 0:TH], in1=bo_sb[:, 0:TH])
            for j in range(HT):
                ffp_quarter(h1_sb, wog[3], 3, j, 0)
                ffp_quarter(h1_sb, wog[3], 3, j, 1)
                nc.gpsimd.tensor_add(out=ff_sb[:, j, TH:DN],
                                     in0=ff_sb[:, j, TH:DN], in1=bo_sb[:, TH:DN])
                tail23_tile(j)

    if compile:
        nc.compile()
    return nc


# ---------------------------------------------------------------- host side

def _pow2_scale(a, target=112.0):
    m = float(np.abs(a).max())
    if m == 0.0:
        return 1.0
    return float(2.0 ** math.floor(math.log2(target / m)))


def _q8(a, s):
    return np.clip(np.asarray(a, np.float32) * s, -224.0, 224.0).astype(NPF8)


def compute_scales(Wq1, Wk1, Wv1, Wo1, Wq2, Wk2, Wv2, Wo2, W_in, W_out):
    inv = np.float32(1.0 / np.sqrt(np.float32(DK)))
    return dict(
        wq1=_pow2_scale(Wq1 * inv), wk1=_pow2_scale(Wk1),
        wv1=_pow2_scale(Wv1), wo1=_pow2_scale(Wo1),
        wq2=_pow2_scale(Wq2 * inv), wk2=_pow2_scale(Wk2),
        wv2=_pow2_scale(Wv2), wo2=_pow2_scale(Wo2),
    )


def pack_inputs(cfg, scales, x, y, Wq1, Wk1, Wv1, Wo1, Wq2, Wk2, Wv2, Wo2,
                W_in, b_in, W_out, b_out):
    Tq, DN, ND, NF = cfg.T, cfg.D, cfg.ND, cfg.NF
    NH = H // 2
    TH = cfg.TH

    def tr8(a):
        return _q8(np.ascontiguousarray(
            a.T.reshape(ND, P, Tq).transpose(1, 0, 2)), S_Y)

    def trbf(a):  # [TH, D] -> [P, ND, TH] bf16 true scale
        return np.ascontiguousarray(
            a.T.reshape(ND, P, TH).transpose(1, 0, 2)).astype(NPBF16)

    def qk_pack(W, h0, s):
        Wh = W[h0:h0 + NH]
        Wp = Wh.reshape(NH // 2, 2, DN, DK).transpose(2, 0, 1, 3)
        Wp = Wp.reshape(DN, NH * DK)
        return _q8(np.ascontiguousarray(
            Wp.reshape(ND, P, NH * DK).transpose(1, 0, 2)), s)

    def v_pack(W, h0, s):
        Wh = W[h0:h0 + NH].transpose(1, 0, 2).reshape(DN, NH * DV)
        return _q8(np.ascontiguousarray(
            Wh.reshape(ND, P, NH * DV).transpose(1, 0, 2)), s)

    def wo_pack(Wo, h, s):
        Ws = Wo[NH * DV * h:NH * DV * h + NH * DV]
        return _q8(np.ascontiguousarray(
            Ws.reshape(4, P, DN).transpose(1, 0, 2)), s)

    def wi_pack(W_in):  # full FF -> [NF, P, ND, P] bf16 chunked lhsT[d, f]
        A = np.asarray(W_in, np.float32).T.reshape(ND, P, NF, P)
        return np.ascontiguousarray(A.transpose(2, 1, 0, 3)).astype(NPBF16)

    def wot_pack(W_out):
        Ws = np.asarray(W_out, np.float32).T      # [FF, D]
        return np.ascontiguousarray(
            Ws.reshape(NF, P, DN).transpose(1, 0, 2)).astype(NPBF16)

    inv = np.float32(1.0 / np.sqrt(np.float32(DK)))
    wi_b = wi_pack(W_in)
    wot_b = wot_pack(W_out)
    bi_b = np.ascontiguousarray(
        np.asarray(b_in, np.float32).reshape(NF, P).T)
    in_maps = []
    for c in range(2 * x.shape[0]):
        b, h = c // 2, c % 2
        h0 = NH * h
        in_maps.append(dict(
            yT=tr8(y[b]),
            yTo=trbf(y[b, h * TH:(h + 1) * TH]),
            xT=tr8(x[b]),
            ynat=np.ascontiguousarray(y[b, h * TH:(h + 1) * TH]).astype(np.float32),
            wq1=qk_pack(Wq1 * inv, h0, scales["wq1"]),
            wk1=qk_pack(Wk1, h0, scales["wk1"]),
            wv1=v_pack(Wv1, h0, scales["wv1"]),
            wo1=wo_pack(Wo1, h, scales["wo1"]),
            wq2=qk_pack(Wq2 * inv, h0, scales["wq2"]),
            wk2=qk_pack(Wk2, h0, scales["wk2"]),
            wv2=v_pack(Wv2, h0, scales["wv2"]),
            wo2=wo_pack(Wo2, h, scales["wo2"]),
            wi=wi_b, wot=wot_b, bi=bi_b,
            bo=np.asarray(b_out).reshape(1, DN).astype(np.float32),
        ))
    return in_maps


_PROG_CACHE = {}


def kernel(**inputs) -> np.ndarray:
    cfg = Cfg()
    inputs = {k: np.asarray(v, np.float32) for k, v in inputs.items()}
    scales = compute_scales(
        inputs["Wq1"], inputs["Wk1"], inputs["Wv1"], inputs["Wo1"],
        inputs["Wq2"], inputs["Wk2"], inputs["Wv2"], inputs["Wo2"],
        inputs["W_in"], inputs["W_out"])
    key = tuple(sorted(scales.items()))
    if key not in _PROG_CACHE:
        _PROG_CACHE[key] = build_program(cfg, scales)
    nc = _PROG_CACHE[key]
    in_maps = pack_inputs(cfg, scales, **inputs)
    res = run_bass_kernel_spmd(nc, in_maps, core_ids=list(range(8)))
    TH = cfg.TH
    out = np.empty((B, T, D), np.float32)
    for c in range(8):
        b, h = c // 2, c % 2
        out[b, h * TH:(h + 1) * TH] = res.results[c]["out"]
    return out


# revision 15
# speedup vs baseline: 1.1423x; 1.0061x over previous
"""Trainium2 Bass kernel for nn_DecoderStack — v4.

Sharding over 8 NeuronCores: core c -> batch b=c//2, half h=c%2.
MHA1/MHA2: head-split (8 heads/core), pair ReduceScatter after Wo (chunked
4x[256,D] bf16, pipelined with tails).  FFN: t-split (full FF=4096 over the
core's own 512 rows), entirely bf16 — no FFN collective.

fp8e4 DoubleRow matmuls: QKV projections, attnV (V pre-scaled by 1/denom),
Wo.  bf16: scores (K=64) and the whole FFN (fp8 there costs too much
accuracy).  Dequant scales are exact powers of two folded into the Exp
activation scale and PSUM-eviction multiplies.

Engine split: PE matmuls; scalar exp (the head-loop pacer) + sqrt; vector
qk/v/pt/wvp evictions + tails; pool (gpsimd) wo/h1/ffp evictions.

Schedule (PE kept dense to hold the 2.4GHz p-state):
  P0: mha1 Q/K/V projection units
  P1: mha1 head loop; filler: 3 mha2-projection units + 1 FFN-in chunk/head
  P2: Wo1 tiles -> chunked RS0 ; FFN-in chunks fc=8..31
  P3: mha2 head loop; filler: FFN-out quarters (wot groups 0-1) + tail1
  P4: Wo2 tiles (k,k+4) -> RS1 chunk k ; FFN-out groups 2-3 between chunks;
      fused tail2+tail3 per chunk
"""

import sys

for _p in ("/opt/trn_rl_repo", "/root/.axon_site"):
    if _p not in sys.path:
        sys.path.insert(0, _p)

import contextlib
import math

import numpy as np

import concourse.bass as bass
import concourse.bacc as bacc
import concourse.tile as tile
from concourse import mybir
from concourse.bass_utils import run_bass_kernel_spmd

B, T, D, H, DK, DV, FF = 4, 1024, 1024, 16, 64, 64, 4096
P = 128
FP32 = mybir.dt.float32
BF16 = mybir.dt.bfloat16
FP8 = mybir.dt.float8e4
NPBF16 = mybir.dt.np(BF16)
NPF8 = mybir.dt.np(FP8)
DR = mybir.MatmulPerfMode.DoubleRow

# quantization scales (exact powers of two)
S_Y = 32.0          # y, x inputs (sigma ~1)
S_ATT = 4096.0      # attn-partial psum scale (baked into wvp)
S_PT = 32.0         # pt fp8 scale
S_WVA = 16.0        # wv_all fp8 storage scale


class Cfg:
    def __init__(self, T_=T, D_=D, FF_=FF):
        self.T = T_
        self.D = D_
        self.FF = FF_
        self.NT = T_ // P          # 8 t/s tiles
        self.ND = D_ // P          # 8 d chunks
        self.HT = T_ // 2 // P     # 4 own-row tiles
        self.NF = FF_ // P         # 32 f chunks (full FF)
        self.NG = 4                # wot fc-groups
        self.GF = self.NF // self.NG   # 8 fc per group
        self.HK = 8 * DK           # 512
        self.HV = 8 * DV           # 512
        self.TH = T_ // 2          # 512 own rows


def build_program(cfg: Cfg, scales: dict, n_cores: int = 8, compile: bool = True):
    nc = bacc.Bacc("TRN2", target_bir_lowering=False, debug=False,
                   num_devices=n_cores)
    NT, ND, NF, HT, TH = cfg.NT, cfg.ND, cfg.NF, cfg.HT, cfg.TH
    NG, GF = cfg.NG, cfg.GF
    Tq, DN = cfg.T, cfg.D
    NTH = 2
    NDH = DN // TH

    EXP_SCALE = (1.0 / (S_Y * S_Y * scales["wq1"] * scales["wk1"]),
                 1.0 / (S_Y * S_Y * scales["wq2"] * scales["wk2"]))
    ATT_M = (S_ATT / S_WVA, S_ATT / S_WVA)
    V_EVICT = (S_WVA / (S_Y * scales["wv1"]), S_WVA / (S_Y * scales["wv2"]))
    PT_EVICT = S_PT / S_ATT
    WO_EVICT = (1.0 / (S_PT * scales["wo1"]), 1.0 / (S_PT * scales["wo2"]))

    def dram_in(name, shape, dt=FP8):
        return nc.dram_tensor(name, shape, dt, kind="ExternalInput")

    yT = dram_in("yT", [P, ND, Tq])
    yTo = dram_in("yTo", [P, ND, TH], BF16)    # own t-half, true scale
    xT = dram_in("xT", [P, ND, Tq])
    ynat = dram_in("ynat", [TH, DN], FP32)
    wq1 = dram_in("wq1", [P, ND, cfg.HK])
    wk1 = dram_in("wk1", [P, ND, cfg.HK])
    wv1 = dram_in("wv1", [P, ND, cfg.HV])
    wo1 = dram_in("wo1", [P, 4, DN])
    wq2 = dram_in("wq2", [P, ND, cfg.HK])
    wk2 = dram_in("wk2", [P, ND, cfg.HK])
    wv2 = dram_in("wv2", [P, ND, cfg.HV])
    wo2 = dram_in("wo2", [P, 4, DN])
    wi = dram_in("wi", [NF, P, ND, P], BF16)   # W_in chunks, lhsT[d, f]
    wot = dram_in("wot", [P, NF, DN], BF16)
    bi = dram_in("bi", [P, NF], FP32)
    bo = dram_in("bo", [1, DN], FP32)
    out = nc.dram_tensor("out", [TH, DN], FP32, kind="ExternalOutput")

    with tile.TileContext(nc) as tc:
        with contextlib.ExitStack() as ctx:
            p1 = ctx.enter_context(tc.tile_pool(name="p1", bufs=1))
            xw = ctx.enter_context(tc.tile_pool(name="xw", bufs=2))
            qkt = ctx.enter_context(tc.tile_pool(name="qkt", bufs=2))
            ptp = ctx.enter_context(tc.tile_pool(name="ptp", bufs=1))
            expp = ctx.enter_context(tc.tile_pool(name="expp", bufs=8))
            wvpp = ctx.enter_context(tc.tile_pool(name="wvpp", bufs=2))
            wic = ctx.enter_context(tc.tile_pool(name="wic", bufs=2))
            rows = ctx.enter_context(tc.tile_pool(name="rows", bufs=2))
            small = ctx.enter_context(tc.tile_pool(name="small", bufs=2))
            psum = ctx.enter_context(tc.tile_pool(name="psum", bufs=2, space="PSUM"))
            psc = ctx.enter_context(tc.tile_pool(name="psc", bufs=2, space="PSUM"))
            ppp = ctx.enter_context(tc.tile_pool(name="ppp", bufs=1, space="PSUM"))
            dram = ctx.enter_context(tc.tile_pool(name="dram", bufs=1, space="DRAM"))

            # chunked RS staging: both MHAs: 2x in [512, DN] -> out [256, DN]
            bnc_in = [[dram.tile([4 * P, DN], BF16, tag=f"bi{m}{k}",
                                 name=f"bi{m}{k}") for k in range(2)]
                      for m in range(2)]
            bnc_out = [[dram.tile([2 * P, DN], BF16, tag=f"bo{m}{k}",
                                  name=f"bo{m}{k}") for k in range(2)]
                       for m in range(2)]

            # ---------------- persistent loads (order matters for startup)
            yT_sb = p1.tile([P, ND, Tq], FP8, tag="yT")
            nc.sync.dma_start(yT_sb[:], yT[:])
            wq1_sb = p1.tile([P, ND, cfg.HK], FP8, tag="wq1")
            wk1_sb = p1.tile([P, ND, cfg.HK], FP8, tag="wk1")
            for pr in range(4):
                nc.sync.dma_start(wq1_sb[:, :, pr * P:(pr + 1) * P],
                                  wq1[:, :, pr * P:(pr + 1) * P])
                nc.sync.dma_start(wk1_sb[:, :, pr * P:(pr + 1) * P],
                                  wk1[:, :, pr * P:(pr + 1) * P])
            bi_sb = p1.tile([P, NF], FP32, tag="bi")
            nc.sync.dma_start(bi_sb[:], bi[:])
            bo_sb = p1.tile([P, DN], FP32, tag="bo")
            bo_ap = bo[:]
            nc.sync.dma_start(
                bo_sb[:],
                bass.AP(tensor=bo_ap.tensor, offset=bo_ap.offset,
                        ap=[[0, P]] + list(bo_ap.ap[1:])))

            nsub = max(1, DN // 512)
            sub = DN // nsub

            def sub_norm(x_sb):
                stats = small.tile([P, nsub, 6], FP32, tag="stats")
                for i in range(nsub):
                    nc.vector.bn_stats(
                        out=stats[:, i, :], in_=x_sb[:, i * sub:(i + 1) * sub])
                mv = small.tile([P, 2], FP32, tag="mv")
                nc.vector.bn_aggr(out=mv[:], in_=stats[:])
                std = small.tile([P, 1], FP32, tag="std")
                nc.scalar.activation(
                    out=std[:], in_=mv[:, 1:2],
                    func=mybir.ActivationFunctionType.Sqrt,
                    scale=float(DN) / float(DN - 1))
                msum = small.tile([P, 1], FP32, tag="msum")
                nc.vector.tensor_add(out=msum[:], in0=mv[:, 0:1], in1=std[:])
                nc.vector.tensor_scalar_sub(out=x_sb[:], in0=x_sb[:],
                                            scalar1=msum[:])

            # SBUF-resident out1 and ff (true scale)
            out1_sb = p1.tile([P, HT, DN], BF16, tag="out1")
            ff_sb = p1.tile([P, HT, DN], BF16, tag="ff")

            def tail1_tile(j):
                # out1 rows j: sub_norm(m1 + y) -> out1_sb
                t = rows.tile([P, DN], FP32, tag="rows", name=f"t1_{j}")
                nc.sync.dma_start(t[:], ynat[j * P:(j + 1) * P, :])
                tb = rows.tile([P, DN], BF16, tag="rowsb", name=f"t1b_{j}")
                nc.sync.dma_start(
                    tb[:], bnc_out[0][j // 2][(j % 2) * P:(j % 2) * P + P, :])
                nc.gpsimd.tensor_add(out=t[:], in0=t[:], in1=tb[:])
                sub_norm(t)
                nc.vector.tensor_copy(out=out1_sb[:, j, :], in_=t[:])

            def tail23_tile(j):
                # out2 = sub_norm(out1 + m2); out = sub_norm(ff + out2 + bo)
                m2b = rows.tile([P, DN], BF16, tag="rowsb", name=f"m2b_{j}")
                nc.sync.dma_start(
                    m2b[:], bnc_out[1][j // 2][(j % 2) * P:(j % 2) * P + P, :])
                o2 = rows.tile([P, DN], FP32, tag="rows", name=f"o2_{j}")
                nc.gpsimd.tensor_add(out=o2[:], in0=out1_sb[:, j, :], in1=m2b[:])
                sub_norm(o2)
                nc.vector.tensor_add(out=o2[:], in0=o2[:], in1=ff_sb[:, j, :])
                sub_norm(o2)
                nc.sync.dma_start(out[j * P:(j + 1) * P, :], o2[:])

            def reduce_chunk2(m, k):
                nc.gpsimd.collective_compute(
                    "ReduceScatter",
                    mybir.AluOpType.add,
                    replica_groups=[[2 * g, 2 * g + 1]
                                    for g in range(n_cores // 2)],
                    ins=[bnc_in[m][k].opt()],
                    outs=[bnc_out[m][k].opt()])

            def reduce_chunk(m, k):
                nc.gpsimd.collective_compute(
                    "ReduceScatter",
                    mybir.AluOpType.add,
                    replica_groups=[[2 * g, 2 * g + 1]
                                    for g in range(n_cores // 2)],
                    ins=[bnc_in[m][k].opt()],
                    outs=[bnc_out[m][k].opt()])

            # ---------------- building blocks
            def load_w(tag, src, shape, dt=FP8):
                t = p1.tile(shape, dt, tag=tag)
                nc.sync.dma_start(t[:], src[:])
                return t

            def qk_unit(dst, w_sb, kvT_sb, pair, th):
                tsl = slice(th * TH, (th + 1) * TH)
                pq = psum.tile([P, TH], FP32, tag="mm")
                for c in range(ND // 2):
                    nc.tensor.matmul(
                        pq[:], lhsT=w_sb[:, 2 * c:2 * c + 2,
                                         pair * P:(pair + 1) * P],
                        rhs=kvT_sb[:, 2 * c:2 * c + 2, tsl],
                        start=(c == 0), stop=(c == ND // 2 - 1),
                        perf_mode=DR)
                nc.vector.tensor_copy(out=dst[:, pair, tsl], in_=pq[:])

            def v_unit(wv_all, wv_sb, kvT_sb, st, v_evict):
                pv = psum.tile([P, cfg.HV], FP32, tag="mm")
                for c in range(ND // 2):
                    nc.tensor.matmul(
                        pv[:], lhsT=kvT_sb[:, 2 * c:2 * c + 2,
                                           st * P:(st + 1) * P],
                        rhs=wv_sb[:, 2 * c:2 * c + 2, :],
                        start=(c == 0), stop=(c == ND // 2 - 1),
                        perf_mode=DR)
                nc.vector.tensor_scalar_mul(out=wv_all[:, st, :], in0=pv[:],
                                            scalar1=v_evict)

            def head_loop(mi, wqt_sb, wkt_sb, wv_all, pt_sb, units):
                """scores/exp of head h interleaved with attnV of head h-1.

                `units` is a list of (fn, args) filler closures; one is issued
                after each attnV pair-step so the in-order PE stream always
                has independent work queued while scores wait on the scalar
                exp pipeline (keeps the PE p-state at 2.4GHz).
                """
                uq = list(units)
                nslots = 8 * (NT // 2 + 1)
                state = {}

                def pop_units(slot):
                    want = len(units) * (slot + 1) // nslots
                    done = len(units) - len(uq)
                    for _ in range(want - done):
                        fn, args = uq.pop(0)
                        fn(*args)
                exp_scale = EXP_SCALE[mi - 1]
                att_m = ATT_M[mi - 1]

                def partial_prelude(h):
                    exps, denom = state.pop(h)
                    rden = small.tile([P, NT], FP32, tag="rden")
                    nc.vector.reciprocal(out=rden[:], in_=denom[:])
                    nc.vector.tensor_scalar_mul(out=rden[:], in0=rden[:],
                                                scalar1=att_m)
                    wvp = wvpp.tile([P, NT, DV], FP8, tag="wvp")
                    for st in range(NT):
                        nc.vector.tensor_scalar_mul(
                            out=wvp[:, st, :],
                            in0=wv_all[:, st, 64 * h:64 * h + 64],
                            scalar1=rden[:, st:st + 1])
                    pa = ppp.tile([64, TH], FP32, tag="pa", name=f"pa{mi}_{h}")
                    pb = ppp.tile([64, TH], FP32, tag="pb", name=f"pb{mi}_{h}")
                    return exps, wvp, pa, pb

                def partial_step(ctx_p, c):
                    exps, wvp, pa, pb = ctx_p
                    e = exps[c]
                    nc.tensor.matmul(
                        pa[:], lhsT=wvp[:, 2 * c:2 * c + 2, :],
                        rhs=e[:, :, 0:TH],
                        start=(c == 0), stop=(c == NT // 2 - 1),
                        perf_mode=DR, skip_group_check=True)
                    nc.tensor.matmul(
                        pb[:], lhsT=wvp[:, 2 * c:2 * c + 2, :],
                        rhs=e[:, :, TH:Tq],
                        start=(c == 0), stop=(c == NT // 2 - 1),
                        perf_mode=DR, skip_group_check=True)

                def partial_evict(ctx_p, h):
                    _, _, pa, pb = ctx_p
                    pair, j = h // 2, h % 2
                    lo, hi = 64 * j, 64 * j + 64
                    nc.vector.tensor_scalar_mul(
                        out=pt_sb[lo:hi, pair, 0:TH], in0=pa[:],
                        scalar1=PT_EVICT)
                    nc.vector.tensor_scalar_mul(
                        out=pt_sb[lo:hi, pair, TH:Tq], in0=pb[:],
                        scalar1=PT_EVICT)

                for h in range(8):
                    pair, j = h // 2, h % 2
                    lo, hi = 64 * j, 64 * j + 64
                    ctx_p = partial_prelude(h - 1) if h > 0 else None
                    denom = small.tile([P, NT], FP32, tag="denom")
                    exps = []
                    for c in range(NT // 2):
                        e = expp.tile([P, 2, Tq], FP8, tag="exp",
                                      name=f"exp{mi}_{h}_{c}")
                        for jj in range(2):
                            st = 2 * c + jj
                            ps = psc.tile([P, Tq], FP32, tag="sc")
                            for th in range(NTH):
                                nc.tensor.matmul(
                                    ps[:, th * TH:(th + 1) * TH],
                                    lhsT=wkt_sb[lo:hi, pair,
                                                st * P:(st + 1) * P],
                                    rhs=wqt_sb[lo:hi, pair,
                                               th * TH:(th + 1) * TH],
                                    start=True, stop=True)
                            nc.scalar.activation(
                                out=e[:, jj, :], in_=ps[:],
                                func=mybir.ActivationFunctionType.Exp,
                                scale=exp_scale,
                                accum_out=denom[:, st:st + 1])
                        exps.append(e)
                        if ctx_p is not None:
                            partial_step(ctx_p, c)
                        pop_units(h * (NT // 2 + 1) + c)
                    if ctx_p is not None:
                        partial_evict(ctx_p, h - 1)
                    state[h] = (exps, denom)
                    pop_units(h * (NT // 2 + 1) + NT // 2)
                ctx_p = partial_prelude(7)
                for c in range(NT // 2):
                    partial_step(ctx_p, c)
                partial_evict(ctx_p, 7)
                while uq:
                    fn, args = uq.pop(0)
                    fn(*args)

            def wo_tile(wo_sb, pt_sb, m, wo_evict, k, half):
                tt = k + HT * half
                mo = rows.tile([P, DN], BF16, tag="rowsb", name=f"mo{m}_{tt}")
                for dh in range(NDH):
                    po = psum.tile([P, TH], FP32, tag="mm")
                    for i2 in range(2):
                        nc.tensor.matmul(
                            po[:],
                            lhsT=pt_sb[:, 2 * i2:2 * i2 + 2,
                                       tt * P:(tt + 1) * P],
                            rhs=wo_sb[:, 2 * i2:2 * i2 + 2,
                                      dh * TH:(dh + 1) * TH],
                            start=(i2 == 0), stop=(i2 == 1),
                            perf_mode=DR)
                    nc.scalar.activation(
                        out=mo[:, dh * TH:(dh + 1) * TH], in_=po[:],
                        func=mybir.ActivationFunctionType.Copy,
                        scale=wo_evict)
                tt2 = k + HT * half
                kk, row = tt2 % 4 // 2, (tt2 % 2) * P + (tt2 // 4) * 2 * P
                nc.sync.dma_start(bnc_in[m][kk][row:row + P, :], mo[:])

            def h1_chunk(h1_sb, yTo_sb, fc):
                wi_c = wic.tile([P, ND, P], BF16, tag="wic", name=f"wic_{fc}")
                nc.sync.dma_start(wi_c[:], wi[fc])
                ph = psum.tile([P, TH], FP32, tag="mm")
                for dc in range(ND):
                    nc.tensor.matmul(
                        ph[:], lhsT=wi_c[:, dc, :],
                        rhs=yTo_sb[:, dc, :],
                        start=(dc == 0), stop=(dc == ND - 1))
                nc.vector.tensor_scalar(
                    out=h1_sb[:, fc, :], in0=ph[:],
                    scalar1=bi_sb[:, fc:fc + 1], scalar2=0.0,
                    op0=mybir.AluOpType.add, op1=mybir.AluOpType.max)

            def ffp_quarter(h1_sb, wog_sb, g, tl, dh):
                # += h1[:, g] @ wot[g] for own-row tile tl, d-half dh
                pf = psum.tile([P, TH], FP32, tag="mm")
                for i in range(GF):
                    nc.tensor.matmul(
                        pf[:], lhsT=h1_sb[:, g * GF + i, tl * P:(tl + 1) * P],
                        rhs=wog_sb[:, i, dh * TH:(dh + 1) * TH],
                        start=(i == 0), stop=(i == GF - 1))
                dst = ff_sb[:, tl, dh * TH:(dh + 1) * TH]
                if g == 0:
                    nc.vector.tensor_copy(out=dst, in_=pf[:])
                else:
                    nc.vector.tensor_add(out=dst, in0=dst, in1=pf[:])

            # ---------------- P0: mha1 Q/K then V projections
            wv1_sb = load_w("wv1", wv1, [P, ND, cfg.HV])
            wqt1 = qkt.tile([P, 4, Tq], BF16, tag="wqt", name="wqt1")
            wkt1 = qkt.tile([P, 4, Tq], BF16, tag="wkt", name="wkt1")
            for pair in range(4):
                for th in range(NTH):
                    qk_unit(wqt1, wq1_sb, yT_sb, pair, th)
                    qk_unit(wkt1, wk1_sb, yT_sb, pair, th)
            wv_all1 = p1.tile([P, NT, cfg.HV], FP8, tag="wv_all1")
            for st in range(NT):
                v_unit(wv_all1, wv1_sb, yT_sb, st, V_EVICT[0])
            # DMAs needed from P1 onward (run behind P0/P1 compute)
            yTo_sb = p1.tile([P, ND, TH], BF16, tag="yTo")
            nc.sync.dma_start(yTo_sb[:], yTo[:])
            xT_sb = xw.tile([P, ND, Tq], FP8, tag="big", name="xT_sb")
            nc.sync.dma_start(xT_sb[:], xT[:])
            wq2_sb = load_w("wq2", wq2, [P, ND, cfg.HK])
            wk2_sb = load_w("wk2", wk2, [P, ND, cfg.HK])
            wv2_sb = load_w("wv2", wv2, [P, ND, cfg.HV])
            wo1_sb = load_w("wo1", wo1, [P, 4, DN])

            # ---------------- P1: mha1 heads + mha2 projections + h1 chunks
            pt1 = ptp.tile([P, 4, Tq], FP8, tag="pt", name="pt1")
            wqt2 = qkt.tile([P, 4, Tq], BF16, tag="wqt", name="wqt2")
            wkt2 = qkt.tile([P, 4, Tq], BF16, tag="wkt", name="wkt2")
            wv_all2 = p1.tile([P, NT, cfg.HV], FP8, tag="wv_all2")
            h1_sb = p1.tile([P, NF, TH], BF16, tag="h1")

            p1_units = []
            for pair in range(4):
                for th in range(NTH):
                    # Q2 projects from y; K2/V2 from x (reference: mha(y,x,x))
                    p1_units.append((qk_unit, (wqt2, wq2_sb, yT_sb, pair, th)))
                    p1_units.append((qk_unit, (wkt2, wk2_sb, xT_sb, pair, th)))
                p1_units.append(
                    (v_unit, (wv_all2, wv2_sb, xT_sb, 2 * pair, V_EVICT[1])))
                p1_units.append(
                    (v_unit, (wv_all2, wv2_sb, xT_sb, 2 * pair + 1, V_EVICT[1])))
                p1_units.append((h1_chunk, (h1_sb, yTo_sb, 2 * pair)))
                p1_units.append((h1_chunk, (h1_sb, yTo_sb, 2 * pair + 1)))

            head_loop(1, wqt1, wkt1, wv_all1, pt1, p1_units)

            # ---------------- P2: Wo1 -> 2-chunk RS0 ; FFN-in fc=8..15
            for k2 in range(2):
                for tt in (2 * k2, 2 * k2 + 1, 2 * k2 + 4, 2 * k2 + 5):
                    wo_tile(wo1_sb, pt1, 0, WO_EVICT[0], tt % 4, tt // 4)
                reduce_chunk2(0, k2)
                for fc in range(8 + 4 * k2, 12 + 4 * k2):
                    h1_chunk(h1_sb, yTo_sb, fc)
            # wot groups 0/1 for P3 (slot A frees when xT dies end-P1)
            wog = [None] * NG
            for g in range(2):
                wog[g] = xw.tile([P, GF, DN], BF16, tag="big", name=f"wog{g}")
                nc.sync.dma_start(wog[g][:], wot[:, g * GF:(g + 1) * GF, :])
            wo2_sb = load_w("wo1", wo2, [P, 4, DN])

            # ---------------- P3: mha2 heads + FFN-out groups 0-1 + tail1
            pt2 = ptp.tile([P, 4, Tq], FP8, tag="pt", name="pt2")

            def load_wog2():
                wog[2] = xw.tile([P, GF, DN], BF16, tag="big", name="wog2")
                nc.sync.dma_start(wog[2][:], wot[:, 2 * GF:3 * GF, :])

            p3_units = []
            for tl in range(HT):
                p3_units.append((ffp_quarter, (h1_sb, wog[0], 0, tl, 0)))
                p3_units.append((ffp_quarter, (h1_sb, wog[0], 0, tl, 1)))
            p3_units.append((load_wog2, ()))
            for tl in range(HT):
                p3_units.append((tail1_tile, (tl,)))
                p3_units.append((ffp_quarter, (h1_sb, wog[1], 1, tl, 0)))
                p3_units.append((ffp_quarter, (h1_sb, wog[1], 1, tl, 1)))

            head_loop(2, wqt2, wkt2, wv_all2, pt2, p3_units)

            # ---------------- P4: Wo2 -> chunked RS1 ; groups 2-3 ; tails
            wog[3] = xw.tile([P, GF, DN], BF16, tag="big", name="wog3")
            nc.sync.dma_start(wog[3][:], wot[:, 3 * GF:4 * GF, :])
            for k in range(HT):
                wo_tile(wo2_sb, pt2, 1, WO_EVICT[1], k, 0)
                wo_tile(wo2_sb, pt2, 1, WO_EVICT[1], k, 1)
                if k % 2 == 1:
                    reduce_chunk2(1, k // 2)
                for fc in range(16 + 2 * k, 18 + 2 * k):
                    h1_chunk(h1_sb, yTo_sb, fc)
            for k in range(HT):
                for fc in range(24 + 2 * k, 26 + 2 * k):
                    h1_chunk(h1_sb, yTo_sb, fc)
                ffp_quarter(h1_sb, wog[2], 2, k, 0)
                ffp_quarter(h1_sb, wog[2], 2, k, 1)
                nc.gpsimd.tensor_add(out=ff_sb[:, k, 0:TH],
                                     in0=ff_sb[:, k, 0:TH], in1=bo_sb[:, 0:TH])
            for j in range(HT):
                ffp_quarter(h1_sb, wog[3], 3, j, 0)
                ffp_quarter(h1_sb, wog[3], 3, j, 1)
                nc.gpsimd.tensor_add(out=ff_sb[:, j, TH:DN],
                                     in0=ff_sb[:, j, TH:DN], in1=bo_sb[:, TH:DN])
                tail23_tile(j)

    if compile:
        nc.compile()
    return nc


# ---------------------------------------------------------------- host side

def _pow2_scale(a, target=112.0):
    m = float(np.abs(a).max())
    if m == 0.0:
        return 1.0
    return float(2.0 ** math.floor(math.log2(target / m)))


def _q8(a, s):
    return np.clip(np.asarray(a, np.float32) * s, -224.0, 224.0).astype(NPF8)


def compute_scales(Wq1, Wk1, Wv1, Wo1, Wq2, Wk2, Wv2, Wo2, W_in, W_out):
    inv = np.float32(1.0 / np.sqrt(np.float32(DK)))
    return dict(
        wq1=_pow2_scale(Wq1 * inv), wk1=_pow2_scale(Wk1),
        wv1=_pow2_scale(Wv1), wo1=_pow2_scale(Wo1),
        wq2=_pow2_scale(Wq2 * inv), wk2=_pow2_scale(Wk2),
        wv2=_pow2_scale(Wv2), wo2=_pow2_scale(Wo2),
    )


def pack_inputs(cfg, scales, x, y, Wq1, Wk1, Wv1, Wo1, Wq2, Wk2, Wv2, Wo2,
                W_in, b_in, W_out, b_out):
    Tq, DN, ND, NF = cfg.T, cfg.D, cfg.ND, cfg.NF
    NH = H // 2
    TH = cfg.TH

    def tr8(a):
        return _q8(np.ascontiguousarray(
            a.T.reshape(ND, P, Tq).transpose(1, 0, 2)), S_Y)

    def trbf(a):  # [TH, D] -> [P, ND, TH] bf16 true scale
        return np.ascontiguousarray(
            a.T.reshape(ND, P, TH).transpose(1, 0, 2)).astype(NPBF16)

    def qk_pack(W, h0, s):
        Wh = W[h0:h0 + NH]
        Wp = Wh.reshape(NH // 2, 2, DN, DK).transpose(2, 0, 1, 3)
        Wp = Wp.reshape(DN, NH * DK)
        return _q8(np.ascontiguousarray(
            Wp.reshape(ND, P, NH * DK).transpose(1, 0, 2)), s)

    def v_pack(W, h0, s):
        Wh = W[h0:h0 + NH].transpose(1, 0, 2).reshape(DN, NH * DV)
        return _q8(np.ascontiguousarray(
            Wh.reshape(ND, P, NH * DV).transpose(1, 0, 2)), s)

    def wo_pack(Wo, h, s):
        Ws = Wo[NH * DV * h:NH * DV * h + NH * DV]
        return _q8(np.ascontiguousarray(
            Ws.reshape(4, P, DN).transpose(1, 0, 2)), s)

    def wi_pack(W_in):  # full FF -> [NF, P, ND, P] bf16 chunked lhsT[d, f]
        A = np.asarray(W_in, np.float32).T.reshape(ND, P, NF, P)
        return np.ascontiguousarray(A.transpose(2, 1, 0, 3)).astype(NPBF16)

    def wot_pack(W_out):
        Ws = np.asarray(W_out, np.float32).T      # [FF, D]
        return np.ascontiguousarray(
            Ws.reshape(NF, P, DN).transpose(1, 0, 2)).astype(NPBF16)

    inv = np.float32(1.0 / np.sqrt(np.float32(DK)))
    wi_b = wi_pack(W_in)
    wot_b = wot_pack(W_out)
    bi_b = np.ascontiguousarray(
        np.asarray(b_in, np.float32).reshape(NF, P).T)
    in_maps = []
    for c in range(2 * x.shape[0]):
        b, h = c // 2, c % 2
        h0 = NH * h
        in_maps.append(dict(
            yT=tr8(y[b]),
            yTo=trbf(y[b, h * TH:(h + 1) * TH]),
            xT=tr8(x[b]),
            ynat=np.ascontiguousarray(y[b, h * TH:(h + 1) * TH]).astype(np.float32),
            wq1=qk_pack(Wq1 * inv, h0, scales["wq1"]),
            wk1=qk_pack(Wk1, h0, scales["wk1"]),
            wv1=v_pack(Wv1, h0, scales["wv1"]),
            wo1=wo_pack(Wo1, h, scales["wo1"]),
            wq2=qk_pack(Wq2 * inv, h0, scales["wq2"]),
            wk2=qk_pack(Wk2, h0, scales["wk2"]),
            wv2=v_pack(Wv2, h0, scales["wv2"]),
            wo2=wo_pack(Wo2, h, scales["wo2"]),
            wi=wi_b, wot=wot_b, bi=bi_b,
            bo=np.asarray(b_out).reshape(1, DN).astype(np.float32),
        ))
    return in_maps


_PROG_CACHE = {}


def kernel(**inputs) -> np.ndarray:
    cfg = Cfg()
    inputs = {k: np.asarray(v, np.float32) for k, v in inputs.items()}
    scales = compute_scales(
        inputs["Wq1"], inputs["Wk1"], inputs["Wv1"], inputs["Wo1"],
        inputs["Wq2"], inputs["Wk2"], inputs["Wv2"], inputs["Wo2"],
        inputs["W_in"], inputs["W_out"])
    key = tuple(sorted(scales.items()))
    if key not in _PROG_CACHE:
        _PROG_CACHE[key] = build_program(cfg, scales)
    nc = _PROG_CACHE[key]
    in_maps = pack_inputs(cfg, scales, **inputs)
    res = run_bass_kernel_spmd(nc, in_maps, core_ids=list(range(8)))
    TH = cfg.TH
    out = np.empty((B, T, D), np.float32)
    for c in range(8):
        b, h = c // 2, c % 2
        out[b, h * TH:(h + 1) * TH] = res.results[c]["out"]
    return out


# revision 16
# speedup vs baseline: 1.1985x; 1.0491x over previous
"""Trainium2 Bass kernel for nn_DecoderStack — v4.

Sharding over 8 NeuronCores: core c -> batch b=c//2, half h=c%2.
MHA1/MHA2: head-split (8 heads/core), pair ReduceScatter after Wo (chunked
4x[256,D] bf16, pipelined with tails).  FFN: t-split (full FF=4096 over the
core's own 512 rows), entirely bf16 — no FFN collective.

fp8e4 DoubleRow matmuls: QKV projections, attnV (V pre-scaled by 1/denom),
Wo.  bf16: scores (K=64) and the whole FFN (fp8 there costs too much
accuracy).  Dequant scales are exact powers of two folded into the Exp
activation scale and PSUM-eviction multiplies.

Engine split: PE matmuls; scalar exp (the head-loop pacer) + sqrt; vector
qk/v/pt/wvp evictions + tails; pool (gpsimd) wo/h1/ffp evictions.

Schedule (PE kept dense to hold the 2.4GHz p-state):
  P0: mha1 Q/K/V projection units
  P1: mha1 head loop; filler: 3 mha2-projection units + 1 FFN-in chunk/head
  P2: Wo1 tiles -> chunked RS0 ; FFN-in chunks fc=8..31
  P3: mha2 head loop; filler: FFN-out quarters (wot groups 0-1) + tail1
  P4: Wo2 tiles (k,k+4) -> RS1 chunk k ; FFN-out groups 2-3 between chunks;
      fused tail2+tail3 per chunk
"""

import sys

for _p in ("/opt/trn_rl_repo", "/root/.axon_site"):
    if _p not in sys.path:
        sys.path.insert(0, _p)

import contextlib
import math

import numpy as np

import concourse.bass as bass
import concourse.bacc as bacc
import concourse.tile as tile
from concourse import mybir
from concourse.bass_utils import run_bass_kernel_spmd

B, T, D, H, DK, DV, FF = 4, 1024, 1024, 16, 64, 64, 4096
P = 128
FP32 = mybir.dt.float32
BF16 = mybir.dt.bfloat16
FP8 = mybir.dt.float8e4
NPBF16 = mybir.dt.np(BF16)
NPF8 = mybir.dt.np(FP8)
DR = mybir.MatmulPerfMode.DoubleRow

# quantization scales (exact powers of two)
S_Y = 32.0          # y, x inputs (sigma ~1)
S_ATT = 4096.0      # attn-partial psum scale (baked into wvp)
S_PT = 32.0         # pt fp8 scale
S_WVA = 16.0        # wv_all fp8 storage scale


class Cfg:
    def __init__(self, T_=T, D_=D, FF_=FF):
        self.T = T_
        self.D = D_
        self.FF = FF_
        self.NT = T_ // P          # 8 t/s tiles
        self.ND = D_ // P          # 8 d chunks
        self.HT = T_ // 2 // P     # 4 own-row tiles
        self.NF = FF_ // P         # 32 f chunks (full FF)
        self.NG = 4                # wot fc-groups
        self.GF = self.NF // self.NG   # 8 fc per group
        self.HK = 8 * DK           # 512
        self.HV = 8 * DV           # 512
        self.TH = T_ // 2          # 512 own rows


def build_program(cfg: Cfg, scales: dict, n_cores: int = 8, compile: bool = True):
    nc = bacc.Bacc("TRN2", target_bir_lowering=False, debug=False,
                   num_devices=n_cores)
    NT, ND, NF, HT, TH = cfg.NT, cfg.ND, cfg.NF, cfg.HT, cfg.TH
    NG, GF = cfg.NG, cfg.GF
    Tq, DN = cfg.T, cfg.D
    NTH = 2
    NDH = DN // TH

    EXP_SCALE = (1.0 / (S_Y * S_Y * scales["wq1"] * scales["wk1"]),
                 1.0 / (S_Y * S_Y * scales["wq2"] * scales["wk2"]))
    ATT_M = (S_ATT / S_WVA, S_ATT / S_WVA)
    V_EVICT = (S_WVA / (S_Y * scales["wv1"]), S_WVA / (S_Y * scales["wv2"]))
    PT_EVICT = S_PT / S_ATT
    WO_EVICT = (1.0 / (S_PT * scales["wo1"]), 1.0 / (S_PT * scales["wo2"]))

    def dram_in(name, shape, dt=FP8):
        return nc.dram_tensor(name, shape, dt, kind="ExternalInput")

    yT = dram_in("yT", [P, ND, Tq])
    yTo = dram_in("yTo", [P, ND, TH], BF16)    # own t-half, true scale
    xT = dram_in("xT", [P, ND, Tq])
    ynat = dram_in("ynat", [TH, DN], FP32)
    wq1 = dram_in("wq1", [P, ND, cfg.HK])
    wk1 = dram_in("wk1", [P, ND, cfg.HK])
    wv1 = dram_in("wv1", [P, ND, cfg.HV])
    wo1 = dram_in("wo1", [P, 4, DN])
    wq2 = dram_in("wq2", [P, ND, cfg.HK])
    wk2 = dram_in("wk2", [P, ND, cfg.HK])
    wv2 = dram_in("wv2", [P, ND, cfg.HV])
    wo2 = dram_in("wo2", [P, 4, DN])
    wi = dram_in("wi", [NF, P, ND, P], BF16)   # W_in chunks, lhsT[d, f]
    wot = dram_in("wot", [P, NF, DN], BF16)
    bi = dram_in("bi", [P, NF], FP32)
    bo = dram_in("bo", [1, DN], FP32)
    out = nc.dram_tensor("out", [TH, DN], FP32, kind="ExternalOutput")

    with tile.TileContext(nc) as tc:
        with contextlib.ExitStack() as ctx:
            p1 = ctx.enter_context(tc.tile_pool(name="p1", bufs=1))
            xw = ctx.enter_context(tc.tile_pool(name="xw", bufs=2))
            qkt = ctx.enter_context(tc.tile_pool(name="qkt", bufs=2))
            ptp = ctx.enter_context(tc.tile_pool(name="ptp", bufs=1))
            expp = ctx.enter_context(tc.tile_pool(name="expp", bufs=8))
            wvpp = ctx.enter_context(tc.tile_pool(name="wvpp", bufs=2))
            wic = ctx.enter_context(tc.tile_pool(name="wic", bufs=2))
            rows = ctx.enter_context(tc.tile_pool(name="rows", bufs=2))
            small = ctx.enter_context(tc.tile_pool(name="small", bufs=2))
            psum = ctx.enter_context(tc.tile_pool(name="psum", bufs=2, space="PSUM"))
            psc = ctx.enter_context(tc.tile_pool(name="psc", bufs=2, space="PSUM"))
            ppp = ctx.enter_context(tc.tile_pool(name="ppp", bufs=1, space="PSUM"))
            dram = ctx.enter_context(tc.tile_pool(name="dram", bufs=1, space="DRAM"))

            # chunked RS staging: both MHAs: 2x in [512, DN] -> out [256, DN]
            bnc_in = [[dram.tile([4 * P, DN], BF16, tag=f"bi{m}{k}",
                                 name=f"bi{m}{k}") for k in range(2)]
                      for m in range(2)]
            bnc_out = [[dram.tile([2 * P, DN], BF16, tag=f"bo{m}{k}",
                                  name=f"bo{m}{k}") for k in range(2)]
                       for m in range(2)]

            # ---------------- persistent loads (order matters for startup)
            yT_sb = p1.tile([P, ND, Tq], FP8, tag="yT")
            nc.sync.dma_start(yT_sb[:], yT[:])
            wq1_sb = p1.tile([P, ND, cfg.HK], FP8, tag="wq1")
            wk1_sb = p1.tile([P, ND, cfg.HK], FP8, tag="wk1")
            for pr in range(4):
                nc.sync.dma_start(wq1_sb[:, :, pr * P:(pr + 1) * P],
                                  wq1[:, :, pr * P:(pr + 1) * P])
                nc.sync.dma_start(wk1_sb[:, :, pr * P:(pr + 1) * P],
                                  wk1[:, :, pr * P:(pr + 1) * P])
            bi_sb = p1.tile([P, NF], FP32, tag="bi")
            nc.sync.dma_start(bi_sb[:], bi[:])
            bo_sb = p1.tile([P, DN], FP32, tag="bo")
            bo_ap = bo[:]
            nc.sync.dma_start(
                bo_sb[:],
                bass.AP(tensor=bo_ap.tensor, offset=bo_ap.offset,
                        ap=[[0, P]] + list(bo_ap.ap[1:])))

            nsub = max(1, DN // 512)
            sub = DN // nsub

            def sub_norm(x_sb):
                stats = small.tile([P, nsub, 6], FP32, tag="stats")
                for i in range(nsub):
                    nc.vector.bn_stats(
                        out=stats[:, i, :], in_=x_sb[:, i * sub:(i + 1) * sub])
                mv = small.tile([P, 2], FP32, tag="mv")
                nc.vector.bn_aggr(out=mv[:], in_=stats[:])
                std = small.tile([P, 1], FP32, tag="std")
                nc.scalar.activation(
                    out=std[:], in_=mv[:, 1:2],
                    func=mybir.ActivationFunctionType.Sqrt,
                    scale=float(DN) / float(DN - 1))
                msum = small.tile([P, 1], FP32, tag="msum")
                nc.vector.tensor_add(out=msum[:], in0=mv[:, 0:1], in1=std[:])
                nc.vector.tensor_scalar_sub(out=x_sb[:], in0=x_sb[:],
                                            scalar1=msum[:])

            # SBUF-resident out1 and ff (true scale)
            out1_sb = p1.tile([P, HT, DN], BF16, tag="out1")
            ff_sb = p1.tile([P, HT, DN], BF16, tag="ff")

            def tail1_tile(j):
                # out1 rows j: sub_norm(m1 + y) -> out1_sb
                t = rows.tile([P, DN], FP32, tag="rows", name=f"t1_{j}")
                nc.sync.dma_start(t[:], ynat[j * P:(j + 1) * P, :])
                tb = rows.tile([P, DN], BF16, tag="rowsb", name=f"t1b_{j}")
                nc.sync.dma_start(
                    tb[:], bnc_out[0][j // 2][(j % 2) * P:(j % 2) * P + P, :])
                nc.gpsimd.tensor_add(out=t[:], in0=t[:], in1=tb[:])
                sub_norm(t)
                nc.vector.tensor_copy(out=out1_sb[:, j, :], in_=t[:])

            def tail23_tile(j):
                # out2 = sub_norm(out1 + m2); out = sub_norm(ff + out2 + bo)
                m2b = rows.tile([P, DN], BF16, tag="rowsb", name=f"m2b_{j}")
                nc.sync.dma_start(
                    m2b[:], bnc_out[1][j // 2][(j % 2) * P:(j % 2) * P + P, :])
                o2 = rows.tile([P, DN], FP32, tag="rows", name=f"o2_{j}")
                nc.gpsimd.tensor_add(out=o2[:], in0=out1_sb[:, j, :], in1=m2b[:])
                sub_norm(o2)
                nc.vector.tensor_add(out=o2[:], in0=o2[:], in1=ff_sb[:, j, :])
                sub_norm(o2)
                nc.sync.dma_start(out[j * P:(j + 1) * P, :], o2[:])

            def reduce_chunk2(m, k):
                nc.gpsimd.collective_compute(
                    "ReduceScatter",
                    mybir.AluOpType.add,
                    replica_groups=[[2 * g, 2 * g + 1]
                                    for g in range(n_cores // 2)],
                    ins=[bnc_in[m][k].opt()],
                    outs=[bnc_out[m][k].opt()])

            def reduce_chunk(m, k):
                nc.gpsimd.collective_compute(
                    "ReduceScatter",
                    mybir.AluOpType.add,
                    replica_groups=[[2 * g, 2 * g + 1]
                                    for g in range(n_cores // 2)],
                    ins=[bnc_in[m][k].opt()],
                    outs=[bnc_out[m][k].opt()])

            # ---------------- building blocks
            def load_w(tag, src, shape, dt=FP8):
                t = p1.tile(shape, dt, tag=tag)
                nc.sync.dma_start(t[:], src[:])
                return t

            def qk_unit(dst, w_sb, kvT_sb, pair, th):
                tsl = slice(th * TH, (th + 1) * TH)
                pq = psum.tile([P, TH], FP32, tag="mm")
                for c in range(ND // 2):
                    nc.tensor.matmul(
                        pq[:], lhsT=w_sb[:, 2 * c:2 * c + 2,
                                         pair * P:(pair + 1) * P],
                        rhs=kvT_sb[:, 2 * c:2 * c + 2, tsl],
                        start=(c == 0), stop=(c == ND // 2 - 1),
                        perf_mode=DR)
                nc.vector.tensor_copy(out=dst[:, pair, tsl], in_=pq[:])

            def v_unit(wv_all, wv_sb, kvT_sb, st, v_evict):
                pv = psum.tile([P, cfg.HV], FP32, tag="mm")
                for c in range(ND // 2):
                    nc.tensor.matmul(
                        pv[:], lhsT=kvT_sb[:, 2 * c:2 * c + 2,
                                           st * P:(st + 1) * P],
                        rhs=wv_sb[:, 2 * c:2 * c + 2, :],
                        start=(c == 0), stop=(c == ND // 2 - 1),
                        perf_mode=DR)
                nc.vector.tensor_scalar_mul(out=wv_all[:, st, :], in0=pv[:],
                                            scalar1=v_evict)

            def head_loop(mi, wqt_sb, wkt_sb, wv_all, pt_sb, units):
                """scores/exp of head h interleaved with attnV of head h-1.

                `units` is a list of (fn, args) filler closures; one is issued
                after each attnV pair-step so the in-order PE stream always
                has independent work queued while scores wait on the scalar
                exp pipeline (keeps the PE p-state at 2.4GHz).
                """
                uq = list(units)
                nslots = 8 * (NT // 2 + 1)
                state = {}

                def pop_units(slot):
                    want = len(units) * (slot + 1) // nslots
                    done = len(units) - len(uq)
                    for _ in range(want - done):
                        fn, args = uq.pop(0)
                        fn(*args)
                exp_scale = EXP_SCALE[mi - 1]
                att_m = ATT_M[mi - 1]

                def partial_prelude(h):
                    exps, denom = state.pop(h)
                    rden = small.tile([P, NT], FP32, tag="rden")
                    nc.vector.reciprocal(out=rden[:], in_=denom[:])
                    nc.vector.tensor_scalar_mul(out=rden[:], in0=rden[:],
                                                scalar1=att_m)
                    wvp = wvpp.tile([P, NT, DV], FP8, tag="wvp")
                    for st in range(NT):
                        nc.vector.tensor_scalar_mul(
                            out=wvp[:, st, :],
                            in0=wv_all[:, st, 64 * h:64 * h + 64],
                            scalar1=rden[:, st:st + 1])
                    pa = ppp.tile([64, TH], FP32, tag="pa", name=f"pa{mi}_{h}")
                    pb = ppp.tile([64, TH], FP32, tag="pb", name=f"pb{mi}_{h}")
                    return exps, wvp, pa, pb

                def partial_step(ctx_p, c):
                    exps, wvp, pa, pb = ctx_p
                    e = exps[c]
                    nc.tensor.matmul(
                        pa[:], lhsT=wvp[:, 2 * c:2 * c + 2, :],
                        rhs=e[:, :, 0:TH],
                        start=(c == 0), stop=(c == NT // 2 - 1),
                        perf_mode=DR, skip_group_check=True)
                    nc.tensor.matmul(
                        pb[:], lhsT=wvp[:, 2 * c:2 * c + 2, :],
                        rhs=e[:, :, TH:Tq],
                        start=(c == 0), stop=(c == NT // 2 - 1),
                        perf_mode=DR, skip_group_check=True)

                def partial_evict(ctx_p, h):
                    _, _, pa, pb = ctx_p
                    pair, j = h // 2, h % 2
                    lo, hi = 64 * j, 64 * j + 64
                    nc.vector.tensor_scalar_mul(
                        out=pt_sb[lo:hi, pair, 0:TH], in0=pa[:],
                        scalar1=PT_EVICT)
                    nc.vector.tensor_scalar_mul(
                        out=pt_sb[lo:hi, pair, TH:Tq], in0=pb[:],
                        scalar1=PT_EVICT)

                for h in range(8):
                    pair, j = h // 2, h % 2
                    lo, hi = 64 * j, 64 * j + 64
                    ctx_p = partial_prelude(h - 1) if h > 0 else None
                    denom = small.tile([P, NT], FP32, tag="denom")
                    exps = []
                    for c in range(NT // 2):
                        e = expp.tile([P, 2, Tq], FP8, tag="exp",
                                      name=f"exp{mi}_{h}_{c}")
                        for jj in range(2):
                            st = 2 * c + jj
                            ps = psc.tile([P, Tq], FP32, tag="sc")
                            for th in range(NTH):
                                nc.tensor.matmul(
                                    ps[:, th * TH:(th + 1) * TH],
                                    lhsT=wkt_sb[lo:hi, pair,
                                                st * P:(st + 1) * P],
                                    rhs=wqt_sb[lo:hi, pair,
                                               th * TH:(th + 1) * TH],
                                    start=True, stop=True)
                            nc.scalar.activation(
                                out=e[:, jj, :], in_=ps[:],
                                func=mybir.ActivationFunctionType.Exp,
                                scale=exp_scale,
                                accum_out=denom[:, st:st + 1])
                        exps.append(e)
                        if ctx_p is not None:
                            partial_step(ctx_p, c)
                        pop_units(h * (NT // 2 + 1) + c)
                    if ctx_p is not None:
                        partial_evict(ctx_p, h - 1)
                    state[h] = (exps, denom)
                    pop_units(h * (NT // 2 + 1) + NT // 2)
                ctx_p = partial_prelude(7)
                for c in range(NT // 2):
                    partial_step(ctx_p, c)
                partial_evict(ctx_p, 7)
                while uq:
                    fn, args = uq.pop(0)
                    fn(*args)

            def wo_tile(wo_sb, pt_sb, m, wo_evict, k, half):
                tt = k + HT * half
                mo = rows.tile([P, DN], BF16, tag="rowsb", name=f"mo{m}_{tt}")
                for dh in range(NDH):
                    po = psum.tile([P, TH], FP32, tag="mm")
                    for i2 in range(2):
                        nc.tensor.matmul(
                            po[:],
                            lhsT=pt_sb[:, 2 * i2:2 * i2 + 2,
                                       tt * P:(tt + 1) * P],
                            rhs=wo_sb[:, 2 * i2:2 * i2 + 2,
                                      dh * TH:(dh + 1) * TH],
                            start=(i2 == 0), stop=(i2 == 1),
                            perf_mode=DR)
                    nc.scalar.activation(
                        out=mo[:, dh * TH:(dh + 1) * TH], in_=po[:],
                        func=mybir.ActivationFunctionType.Copy,
                        scale=wo_evict)
                tt2 = k + HT * half
                kk, row = tt2 % 4 // 2, (tt2 % 2) * P + (tt2 // 4) * 2 * P
                nc.sync.dma_start(bnc_in[m][kk][row:row + P, :], mo[:])

            def h1_chunk(h1_sb, yTo_sb, fc):
                wi_c = wic.tile([P, ND, P], BF16, tag="wic", name=f"wic_{fc}")
                nc.sync.dma_start(wi_c[:], wi[fc])
                ph = psum.tile([P, TH], FP32, tag="mm")
                for dc in range(ND):
                    nc.tensor.matmul(
                        ph[:], lhsT=wi_c[:, dc, :],
                        rhs=yTo_sb[:, dc, :],
                        start=(dc == 0), stop=(dc == ND - 1))
                nc.vector.tensor_scalar(
                    out=h1_sb[:, fc, :], in0=ph[:],
                    scalar1=bi_sb[:, fc:fc + 1], scalar2=0.0,
                    op0=mybir.AluOpType.add, op1=mybir.AluOpType.max)

            def ffp_quarter(h1_sb, wog_sb, g, tl, dh):
                # += h1[:, g] @ wot[g] for own-row tile tl, d-half dh
                pf = psum.tile([P, TH], FP32, tag="mm")
                for i in range(GF):
                    nc.tensor.matmul(
                        pf[:], lhsT=h1_sb[:, g * GF + i, tl * P:(tl + 1) * P],
                        rhs=wog_sb[:, i, dh * TH:(dh + 1) * TH],
                        start=(i == 0), stop=(i == GF - 1))
                dst = ff_sb[:, tl, dh * TH:(dh + 1) * TH]
                if g == 0:
                    nc.vector.tensor_copy(out=dst, in_=pf[:])
                else:
                    nc.vector.tensor_add(out=dst, in0=dst, in1=pf[:])

            # ---------------- P0: mha1 Q/K then V projections
            wv1_sb = load_w("wv1", wv1, [P, ND, cfg.HV])
            wqt1 = qkt.tile([P, 4, Tq], BF16, tag="wqt", name="wqt1")
            wkt1 = qkt.tile([P, 4, Tq], BF16, tag="wkt", name="wkt1")
            for pair in range(4):
                for th in range(NTH):
                    qk_unit(wqt1, wq1_sb, yT_sb, pair, th)
                    qk_unit(wkt1, wk1_sb, yT_sb, pair, th)
            wv_all1 = p1.tile([P, NT, cfg.HV], FP8, tag="wv_all1")
            for st in range(NT):
                v_unit(wv_all1, wv1_sb, yT_sb, st, V_EVICT[0])
            # DMAs needed from P1 onward (run behind P0/P1 compute)
            yTo_sb = p1.tile([P, ND, TH], BF16, tag="yTo")
            nc.sync.dma_start(yTo_sb[:], yTo[:])
            xT_sb = xw.tile([P, ND, Tq], FP8, tag="big", name="xT_sb")
            nc.sync.dma_start(xT_sb[:], xT[:])
            wq2_sb = load_w("wq2", wq2, [P, ND, cfg.HK])
            wk2_sb = load_w("wk2", wk2, [P, ND, cfg.HK])
            wv2_sb = load_w("wv2", wv2, [P, ND, cfg.HV])
            wo1_sb = load_w("wo1", wo1, [P, 4, DN])

            # ---------------- P1: mha1 heads + mha2 projections + h1 chunks
            pt1 = ptp.tile([P, 4, Tq], FP8, tag="pt", name="pt1")
            wqt2 = qkt.tile([P, 4, Tq], BF16, tag="wqt", name="wqt2")
            wkt2 = qkt.tile([P, 4, Tq], BF16, tag="wkt", name="wkt2")
            wv_all2 = p1.tile([P, NT, cfg.HV], FP8, tag="wv_all2")
            h1_sb = p1.tile([P, NF, TH], BF16, tag="h1")

            p1_units = []
            for pair in range(4):
                for th in range(NTH):
                    # Q2 projects from y; K2/V2 from x (reference: mha(y,x,x))
                    p1_units.append((qk_unit, (wqt2, wq2_sb, yT_sb, pair, th)))
                    p1_units.append((qk_unit, (wkt2, wk2_sb, xT_sb, pair, th)))
                p1_units.append(
                    (v_unit, (wv_all2, wv2_sb, xT_sb, 2 * pair, V_EVICT[1])))
                p1_units.append(
                    (v_unit, (wv_all2, wv2_sb, xT_sb, 2 * pair + 1, V_EVICT[1])))
                p1_units.append((h1_chunk, (h1_sb, yTo_sb, 2 * pair)))
                p1_units.append((h1_chunk, (h1_sb, yTo_sb, 2 * pair + 1)))

            head_loop(1, wqt1, wkt1, wv_all1, pt1, p1_units)

            # ---------------- P2: Wo1 -> 2-chunk RS0 ; FFN-in fc=8..15
            for k2 in range(2):
                for tt in (2 * k2, 2 * k2 + 1, 2 * k2 + 4, 2 * k2 + 5):
                    wo_tile(wo1_sb, pt1, 0, WO_EVICT[0], tt % 4, tt // 4)
                reduce_chunk2(0, k2)
                for fc in range(8 + 4 * k2, 12 + 4 * k2):
                    h1_chunk(h1_sb, yTo_sb, fc)
            # wot groups 0/1 for P3 (slot A frees when xT dies end-P1)
            wog = [None] * NG
            for g in range(2):
                wog[g] = xw.tile([P, GF, DN], BF16, tag="big", name=f"wog{g}")
                nc.sync.dma_start(wog[g][:], wot[:, g * GF:(g + 1) * GF, :])
            wo2_sb = load_w("wo1", wo2, [P, 4, DN])

            # ---------------- P3: mha2 heads + FFN-out groups 0-1 + tail1
            pt2 = ptp.tile([P, 4, Tq], FP8, tag="pt", name="pt2")

            def load_wog2():
                wog[2] = xw.tile([P, GF, DN], BF16, tag="big", name="wog2")
                nc.sync.dma_start(wog[2][:], wot[:, 2 * GF:3 * GF, :])

            p3_units = []
            for tl in range(HT):
                p3_units.append((h1_chunk, (h1_sb, yTo_sb, 16 + tl)))
                p3_units.append((ffp_quarter, (h1_sb, wog[0], 0, tl, 0)))
                p3_units.append((ffp_quarter, (h1_sb, wog[0], 0, tl, 1)))
            p3_units.append((load_wog2, ()))
            for tl in range(HT):
                p3_units.append((h1_chunk, (h1_sb, yTo_sb, 20 + tl)))
                p3_units.append((tail1_tile, (tl,)))
                p3_units.append((ffp_quarter, (h1_sb, wog[1], 1, tl, 0)))
                p3_units.append((ffp_quarter, (h1_sb, wog[1], 1, tl, 1)))

            head_loop(2, wqt2, wkt2, wv_all2, pt2, p3_units)

            # ---------------- P4: Wo2 -> chunked RS1 ; groups 2-3 ; tails
            wog[3] = xw.tile([P, GF, DN], BF16, tag="big", name="wog3")
            nc.sync.dma_start(wog[3][:], wot[:, 3 * GF:4 * GF, :])
            for k in range(HT):
                wo_tile(wo2_sb, pt2, 1, WO_EVICT[1], k, 0)
                wo_tile(wo2_sb, pt2, 1, WO_EVICT[1], k, 1)
                if k % 2 == 1:
                    reduce_chunk2(1, k // 2)
                for fc in range(24 + 2 * k, 26 + 2 * k):
                    h1_chunk(h1_sb, yTo_sb, fc)
            for j in range(HT):
                ffp_quarter(h1_sb, wog[2], 2, j, 0)
                ffp_quarter(h1_sb, wog[2], 2, j, 1)
                ffp_quarter(h1_sb, wog[3], 3, j, 0)
                ffp_quarter(h1_sb, wog[3], 3, j, 1)
                nc.gpsimd.tensor_add(out=ff_sb[:, j, :],
                                     in0=ff_sb[:, j, :], in1=bo_sb[:])
                tail23_tile(j)

    if compile:
        nc.compile()
    return nc


# ---------------------------------------------------------------- host side

def _pow2_scale(a, target=112.0):
    m = float(np.abs(a).max())
    if m == 0.0:
        return 1.0
    return float(2.0 ** math.floor(math.log2(target / m)))


def _q8(a, s):
    return np.clip(np.asarray(a, np.float32) * s, -224.0, 224.0).astype(NPF8)


def compute_scales(Wq1, Wk1, Wv1, Wo1, Wq2, Wk2, Wv2, Wo2, W_in, W_out):
    inv = np.float32(1.0 / np.sqrt(np.float32(DK)))
    return dict(
        wq1=_pow2_scale(Wq1 * inv), wk1=_pow2_scale(Wk1),
        wv1=_pow2_scale(Wv1), wo1=_pow2_scale(Wo1),
        wq2=_pow2_scale(Wq2 * inv), wk2=_pow2_scale(Wk2),
        wv2=_pow2_scale(Wv2), wo2=_pow2_scale(Wo2),
    )


def pack_inputs(cfg, scales, x, y, Wq1, Wk1, Wv1, Wo1, Wq2, Wk2, Wv2, Wo2,
                W_in, b_in, W_out, b_out):
    Tq, DN, ND, NF = cfg.T, cfg.D, cfg.ND, cfg.NF
    NH = H // 2
    TH = cfg.TH

    def tr8(a):
        return _q8(np.ascontiguousarray(
            a.T.reshape(ND, P, Tq).transpose(1, 0, 2)), S_Y)

    def trbf(a):  # [TH, D] -> [P, ND, TH] bf16 true scale
        return np.ascontiguousarray(
            a.T.reshape(ND, P, TH).transpose(1, 0, 2)).astype(NPBF16)

    def qk_pack(W, h0, s):
        Wh = W[h0:h0 + NH]
        Wp = Wh.reshape(NH // 2, 2, DN, DK).transpose(2, 0, 1, 3)
        Wp = Wp.reshape(DN, NH * DK)
        return _q8(np.ascontiguousarray(
            Wp.reshape(ND, P, NH * DK).transpose(1, 0, 2)), s)

    def v_pack(W, h0, s):
        Wh = W[h0:h0 + NH].transpose(1, 0, 2).reshape(DN, NH * DV)
        return _q8(np.ascontiguousarray(
            Wh.reshape(ND, P, NH * DV).transpose(1, 0, 2)), s)

    def wo_pack(Wo, h, s):
        Ws = Wo[NH * DV * h:NH * DV * h + NH * DV]
        return _q8(np.ascontiguousarray(
            Ws.reshape(4, P, DN).transpose(1, 0, 2)), s)

    def wi_pack(W_in):  # full FF -> [NF, P, ND, P] bf16 chunked lhsT[d, f]
        A = np.asarray(W_in, np.float32).T.reshape(ND, P, NF, P)
        return np.ascontiguousarray(A.transpose(2, 1, 0, 3)).astype(NPBF16)

    def wot_pack(W_out):
        Ws = np.asarray(W_out, np.float32).T      # [FF, D]
        return np.ascontiguousarray(
            Ws.reshape(NF, P, DN).transpose(1, 0, 2)).astype(NPBF16)

    inv = np.float32(1.0 / np.sqrt(np.float32(DK)))
    wi_b = wi_pack(W_in)
    wot_b = wot_pack(W_out)
    bi_b = np.ascontiguousarray(
        np.asarray(b_in, np.float32).reshape(NF, P).T)
    in_maps = []
    for c in range(2 * x.shape[0]):
        b, h = c // 2, c % 2
        h0 = NH * h
        in_maps.append(dict(
            yT=tr8(y[b]),
            yTo=trbf(y[b, h * TH:(h + 1) * TH]),
            xT=tr8(x[b]),
            ynat=np.ascontiguousarray(y[b, h * TH:(h + 1) * TH]).astype(np.float32),
            wq1=qk_pack(Wq1 * inv, h0, scales["wq1"]),
            wk1=qk_pack(Wk1, h0, scales["wk1"]),
            wv1=v_pack(Wv1, h0, scales["wv1"]),
            wo1=wo_pack(Wo1, h, scales["wo1"]),
            wq2=qk_pack(Wq2 * inv, h0, scales["wq2"]),
            wk2=qk_pack(Wk2, h0, scales["wk2"]),
            wv2=v_pack(Wv2, h0, scales["wv2"]),
            wo2=wo_pack(Wo2, h, scales["wo2"]),
            wi=wi_b, wot=wot_b, bi=bi_b,
            bo=np.asarray(b_out).reshape(1, DN).astype(np.float32),
        ))
    return in_maps


_PROG_CACHE = {}


def kernel(**inputs) -> np.ndarray:
    cfg = Cfg()
    inputs = {k: np.asarray(v, np.float32) for k, v in inputs.items()}
    scales = compute_scales(
        inputs["Wq1"], inputs["Wk1"], inputs["Wv1"], inputs["Wo1"],
        inputs["Wq2"], inputs["Wk2"], inputs["Wv2"], inputs["Wo2"],
        inputs["W_in"], inputs["W_out"])
    key = tuple(sorted(scales.items()))
    if key not in _PROG_CACHE:
        _PROG_CACHE[key] = build_program(cfg, scales)
    nc = _PROG_CACHE[key]
    in_maps = pack_inputs(cfg, scales, **inputs)
    res = run_bass_kernel_spmd(nc, in_maps, core_ids=list(range(8)))
    TH = cfg.TH
    out = np.empty((B, T, D), np.float32)
    for c in range(8):
        b, h = c // 2, c % 2
        out[b, h * TH:(h + 1) * TH] = res.results[c]["out"]
    return out
